# revision 48
# baseline (speedup 1.0000x reference)
"""Trainium2 Bass kernel for nn_AttentionBlock (GroupNorm + 4-head attention + proj + residual).

Sharding: 8 cores = (batch b in 0..3) x (head-pair p in 0..1).  Each core gets
x[b] and the weight slices for its two heads, computes GroupNorm -> QKV ->
attention -> partial proj (over its 128 attention-output channels), and returns
a partial [256, 4096] projection output.  The host sums the two partials per
batch, adds proj bias + residual, and reshapes.

The device program is identical on all cores (SPMD); all per-core variation is
carried by the input tensors.

Layout notes (per core):
  h   = groupnorm(x[b])                SBUF [c=128 x2, n=4096]   (in-place on x)
  Q2  = Wq_pair^T . h                  SBUF [128 (2 heads x 64 d), 4096]
  K2  = Wk_pair^T . h                  SBUF [128, 4096]
  V1  = [h^T . Wv_pair | ones]         SBUF [128 (m-chunk), 32, 2, 65]
  scores^T[m, n] = sum_d K[d,m] Q[d,n] via matmul(lhsT=K2[64 rows], rhs=Q2),
     two heads packed concurrently in PE row-groups (0,0) and (64,0).
  pexp = exp(0.125 * scores^T)         (ScalarE, no max-subtraction)
  pv[d+1, n] += V1[m-chunk]^T . pexp   accumulated over 32 m-chunks in PSUM;
     row 64 (the ones column) accumulates the softmax denominator.
  A = pv[0:64] * (1/pv[64]) broadcast  -> attention output [c, n] per head
  y_partial = Wp[:, pair]^T . A        [256, 4096] -> DRAM
"""

import numpy as np

import concourse.bacc as bacc
import concourse.bass as bass
import concourse.mybir as mybir
import concourse.tile as tile

B = 4
C = 256
N = 4096          # 64*64
NH = 4
D = 64            # head dim
GROUPS = 32
EPS = 1e-5
NCORES = 8
SCALE = float(D) ** -0.5  # 0.125
SCHRA_A = SCALE * (2.0 ** 23) / float(np.log(2.0))
# +2^15 rounds-to-nearest when the bf16 high half is extracted
SCHRA_B = float(127 * 2 ** 23 - 545947 + 32768)
F32 = mybir.dt.float32

MB = 32           # m chunks of 128
NB = 8            # n chunks of 512

F32R = mybir.dt.float32r


def _cast(ap, use_f32r):
    """Vestigial: tiles are already declared in the matmul dtype."""
    return ap


def _build_program(has_v_bias: bool, f32r_attn: bool = False, f32r_mm: bool = False, chain: int = 1, ablate_dma: bool = False, ablate2: str = ''):
    nc = bacc.Bacc("TRN2", target_bir_lowering=False)
    MMD = F32R if f32r_mm else F32      # dtype of tiles feeding qkv/proj matmuls
    if f32r_mm == "bf16":
        # f32r runs the qkv/proj matmuls in 2-pass LOW_HIGH mode (~2x the
        # streaming time); bf16 operands run single-pass.
        MMD = mybir.dt.bfloat16
    MAD = F32R if f32r_attn else F32    # dtype of lhsT tiles for QK/PV (K2, V1)
    MADR = MAD                          # dtype of rhs tiles for QK/PV (Q2, pexp)
    if f32r_attn == "bf16":
        MAD = MADR = mybir.dt.bfloat16
    elif f32r_attn == "mixed":
        MAD = F32R
        MADR = mybir.dt.bfloat16
    QKD = MAD   # dtype of Q2/K2 (the QK^T matmul operands)
    PVD = MADR  # dtype of V1/pexp (the PV matmul operands)
    if f32r_attn == "qk_bf16":
        # QK^T in bf16 (fast weight load on the K chunks); PV stays f32r so
        # the ACT exp writes pexp at fp32 speed (bf16 ACT writes measured
        # ~17% slower).  Matmul operand dtypes must match pairwise (walrus
        # verifier), so the split is per-matmul, not per-operand-side.
        QKD = mybir.dt.bfloat16
        PVD = F32R

    xb = nc.dram_tensor("xb", [C, N], F32, kind="ExternalInput")
    wqkT = nc.dram_tensor("wqkT", [C, 256], F32, kind="ExternalInput")   # cols 0:128 Qpair, 128:256 Kpair
    wvT = nc.dram_tensor("wvT", [C, 128], F32, kind="ExternalInput")
    wpT = nc.dram_tensor("wpT", [128, C], F32, kind="ExternalInput")     # rows = pair channels
    gnw = nc.dram_tensor("gnw", [C], F32, kind="ExternalInput")
    gnb = nc.dram_tensor("gnb", [C], F32, kind="ExternalInput")
    qkb = nc.dram_tensor("qkb", [256], F32, kind="ExternalInput")        # 0:128 Q bias pair, 128:256 K bias pair
    if has_v_bias:
        vb = nc.dram_tensor("vb", [128], F32, kind="ExternalInput")
    yp = nc.dram_tensor("yp", [C, N], F32, kind="ExternalOutput")

    # group-indicator matrix: G[c, g] = 1 if c // 8 == g else 0  (per c-tile)
    g_host = np.zeros((128, 16), dtype=np.float32)
    for c in range(128):
        g_host[c, c // 8] = 1.0
    g_dram = nc.inline_tensor(g_host, name="gmat")
    g_dram_t = nc.inline_tensor(np.ascontiguousarray(g_host.T), name="gmatT")

    with tile.TileContext(nc) as tc:
        # Pin the activation table to natural_log_exp_and_others (set 6):
        # every ACT func this kernel uses (Exp, Ln, identity) lives in that
        # one set, so the fixpoint table-load pass should see it loaded on
        # all paths and never re-load (otherwise it thrashes exp<->ln sets
        # at every n-block boundary, ~2.6us each).
        nc.scalar.add_instruction(
            mybir.InstLoadActFuncSet(
                name=nc.get_next_instruction_name(),
                ins=[], outs=[], act_func_set_id=6,
            )
        )
        with (
            tc.tile_pool(name="consts", bufs=1) as consts,
            tc.tile_pool(name="xh", bufs=2) as xh_pool,
            tc.tile_pool(name="hp", bufs=2) as hp_pool,
            tc.tile_pool(name="qk_sb", bufs=1) as qk_sb,
            tc.tile_pool(name="v1p", bufs=1) as v1p,
            tc.tile_pool(name="st", bufs=2) as st,
            tc.tile_pool(name="pexpp", bufs=4) as pexpp,
            tc.tile_pool(name="recp", bufs=4) as recp,
            tc.tile_pool(name="drec", bufs=4, space="DRAM") as drec,
            tc.tile_pool(name="rbp", bufs=4) as rbp,
            tc.tile_pool(name="yout", bufs=4) as yout,
        ):
            # ---- x loads first: everything else gates on them (GroupNorm
            # stats), and each dma_start costs ~0.7us of serial issue time
            # on its trigger queue.  Weight/const DMAs trigger on the idle
            # GPSIMD queue so they don't delay the x chunks.
            x_ts = []
            if chain == 1 and not ablate_dma:
                xtrigs = [nc.sync, nc.sync, nc.sync, nc.sync,
                          nc.scalar, nc.scalar, nc.scalar, nc.scalar]
                for t in range(2):
                    x_t = xh_pool.tile([128, N], F32, tag="x", name=f"x{t}")
                    for q4 in range(4):
                        qs = slice(q4 * (N // 4), (q4 + 1) * (N // 4))
                        xtrigs[t * 4 + q4].dma_start(
                            out=x_t[:, qs], in_=xb[t * 128:(t + 1) * 128, qs])
                    x_ts.append(x_t)

            # ---- constants ----
            g_sb = consts.tile([128, 16], F32, tag="gsb")
            nc.sync.dma_start(out=g_sb, in_=g_dram[:, :])
            g_sbT = consts.tile([16, 128], F32, tag="gsbT")
            nc.scalar.dma_start(out=g_sbT, in_=g_dram_t[:, :])
            def load_weight(shape, dram_slice, tag):
                w_raw = consts.tile(shape, F32, tag=tag + "_r", name=tag + "_r")
                load_weight.q = getattr(load_weight, "q", 0) + 1
                (nc.sync if load_weight.q % 2 else nc.scalar).dma_start(
                    out=w_raw, in_=dram_slice)
                if not f32r_mm:
                    return w_raw
                w_c = consts.tile(shape, MMD, tag=tag, name=tag)
                nc.vector.tensor_copy(w_c, w_raw)
                return w_c

            w_qk = [load_weight([128, 256], wqkT[t * 128:(t + 1) * 128, :], f"wqk{t}")
                    for t in range(2)]
            w_v = [load_weight([128, 128], wvT[t * 128:(t + 1) * 128, :], f"wv{t}")
                   for t in range(2)]
            wp_lo = load_weight([64, 256], wpT[0:64, :], "wplo")
            wp_hi = load_weight([64, 256], wpT[64:128, :], "wphi")

            gw_t, gb_t = [], []
            for t in range(2):
                gwt = consts.tile([128, 1], F32, tag=f"gw{t}")
                nc.gpsimd.dma_start(out=gwt, in_=gnw[t * 128:(t + 1) * 128])
                gw_t.append(gwt)
                gbt = consts.tile([128, 1], F32, tag=f"gb{t}")
                nc.gpsimd.dma_start(out=gbt, in_=gnb[t * 128:(t + 1) * 128])
                gb_t.append(gbt)
            bq = consts.tile([128, 1], F32, tag="bq")
            nc.gpsimd.dma_start(out=bq, in_=qkb[0:128])
            bk = consts.tile([128, 1], F32, tag="bk")
            nc.gpsimd.dma_start(out=bk, in_=qkb[128:256])
            if has_v_bias:
                vb_sb = consts.tile([128, 128], F32, tag="vbsb")
                nc.gpsimd.dma_start(
                    out=vb_sb,
                    in_=bass.AP(tensor=vb, offset=0, ap=[[0, 128], [1, 128]]),
                )
            eps16 = consts.tile([16, 1], F32, tag="eps16")
            nc.vector.memset(eps16, EPS)

            for _rep in range(chain):
                # ---- phase A: GroupNorm (in place: x tile becomes h tile) ----
                h_t = []
                stats_t = []
                scale_t, bias_t = [], []
                with tc.tile_pool(name="ps_g", bufs=2, space="PSUM") as ps_g:
                    for t in range(2):
                        if x_ts:
                            x_t = x_ts[t]
                        else:
                            x_t = xh_pool.tile([128, N], F32, tag="x", name=f"x{t}")
                            if not ablate_dma or _rep == 0:
                                for q4 in range(4):
                                    qs = slice(q4 * (N // 4), (q4 + 1) * (N // 4))
                                    nc.sync.dma_start(out=x_t[:, qs],
                                                      in_=xb[t * 128:(t + 1) * 128, qs])
                            else:
                                nc.vector.memset(x_t[:, 0:1], 0.5)

                        stats = st.tile([128, 8, 6], F32, tag="stats",
                                        name=f"stats{t}")
                        stats_t.append(stats)
                        h_t.append(x_t)
                    # interleaved so tile1's stats don't queue behind all of
                    # tile0's on the in-order DVE
                    for k in range(8):
                        for t in range(2):
                            xr = h_t[t].rearrange("p (k f) -> p k f", f=512)
                            nc.vector.bn_stats(out=stats_t[t][:, k, :],
                                               in_=xr[:, k, :])
                    for t in range(2):
                        x_t = h_t[t]
                        stats = stats_t[t]
                        mv = st.tile([128, 2], F32, tag="mv")
                        nc.vector.bn_aggr(out=mv, in_=stats)

                        # mq = [mean_c, var_c + mean_c^2]
                        mq = st.tile([128, 2], F32, tag="mq")
                        nc.vector.tensor_copy(mq[:, 0:1], mv[:, 0:1])
                        sq = st.tile([128, 1], F32, tag="sq")
                        nc.vector.tensor_mul(sq, mv[:, 0:1], mv[:, 0:1])
                        nc.vector.tensor_add(mq[:, 1:2], mv[:, 1:2], sq)

                        gps = ps_g.tile([16, 2], F32, tag="gps")
                        nc.tensor.matmul(gps, lhsT=g_sb, rhs=mq, start=True, stop=True)
                        # per-group E[x], E[x^2]
                        gsb = st.tile([16, 2], F32, tag="gsb2")
                        nc.scalar.mul(gsb, gps, 0.125)
                        sqg = st.tile([16, 1], F32, tag="sqg")
                        nc.vector.tensor_mul(sqg, gsb[:, 0:1], gsb[:, 0:1])
                        var = st.tile([16, 1], F32, tag="var")
                        nc.vector.tensor_sub(var, gsb[:, 1:2], sqg)
                        lv = st.tile([16, 1], F32, tag="lv")
                        nc.scalar.activation(lv, var, mybir.ActivationFunctionType.Ln,
                                             bias=eps16, scale=1.0)
                        rstd = st.tile([16, 1], F32, tag="rstd")
                        nc.scalar.activation(rstd, lv, mybir.ActivationFunctionType.Exp,
                                             scale=-0.5)
                        ms = st.tile([16, 2], F32, tag=f"ms{t}")
                        nc.vector.tensor_copy(ms[:, 0:1], gsb[:, 0:1])
                        nc.vector.tensor_copy(ms[:, 1:2], rstd)

                        # broadcast per-group stats back to per-channel
                        # [128,2] with a tiny matmul on the (otherwise idle)
                        # PE -- two DMA round-trips were ~3us of critical path
                        bps = ps_g.tile([128, 2], F32, tag="bps")
                        nc.tensor.matmul(bps, lhsT=g_sbT, rhs=ms, start=True,
                                         stop=True)
                        bmean = bps[:, 0:1]
                        brstd = bps[:, 1:2]
                        sc = consts.tile([128, 1], F32, tag=f"scale{t}")
                        nc.vector.tensor_mul(sc, brstd, gw_t[t])
                        scale_t.append(sc)
                        tmp = st.tile([128, 1], F32, tag="tmpb")
                        nc.vector.tensor_mul(tmp, bmean, sc)
                        bi = consts.tile([128, 1], F32, tag=f"bias{t}")
                        nc.vector.tensor_sub(bi, gb_t[t], tmp)
                        bias_t.append(bi)

                    x_t_list = h_t
                    h_t = []
                    for t in range(2):
                        # h = x * scale + bias (chunked: downstream QKV
                        # matmuls depend on whole-tile writes)
                        h_new = hp_pool.tile([128, N], MMD, tag="h", name=f"h{t}")
                        for q4 in range(4):
                            qs = slice(q4 * (N // 4), (q4 + 1) * (N // 4))
                            nc.vector.tensor_scalar(
                                out=h_new[:, qs], in0=x_t_list[t][:, qs],
                                scalar1=scale_t[t], scalar2=bias_t[t],
                                op0=mybir.AluOpType.mult, op1=mybir.AluOpType.add,
                            )
                        h_t.append(h_new)

                # ---- phase B: QKV ----
                Q2 = qk_sb.tile([128, N], QKD, tag="q2")
                K2 = qk_sb.tile([128, N], QKD, tag="k2")
                V1 = v1p.tile([128, MB, 2, 65], PVD, tag="v1")
                def emit_qkv_chunk(pool, tag, cnb, qk_i):
                    dst, col0, bias_ap = ((Q2, 0, bq), (K2, 128, bk))[qk_i]
                    ps = pool.tile([128, 512], F32, tag=tag,
                                   name=f"qkv_{cnb}_{qk_i}")
                    ns = slice(cnb * 512, (cnb + 1) * 512)
                    nc.tensor.matmul(ps, lhsT=_cast(w_qk[0][:, col0:col0 + 128], f32r_mm),
                                     rhs=_cast(h_t[0][:, ns], f32r_mm), start=True, stop=False)
                    nc.tensor.matmul(ps, lhsT=_cast(w_qk[1][:, col0:col0 + 128], f32r_mm),
                                     rhs=_cast(h_t[1][:, ns], f32r_mm), start=False, stop=True)
                    if qk_i == 0:
                        # ACT's free affine port does the bias add
                        nc.scalar.activation(
                            dst[:, ns], ps,
                            mybir.ActivationFunctionType.Identity,
                            bias=bias_ap, scale=1.0,
                        )
                    else:
                        nc.vector.tensor_scalar(
                            out=dst[:, ns], in0=ps, scalar1=bias_ap, scalar2=None,
                            op0=mybir.AluOpType.add,
                        )

                with tc.tile_pool(name="ps_qkv", bufs=3, space="PSUM") as ps_qkv:
                    for nb in range(NB):
                        for qk_i in range(2):
                            emit_qkv_chunk(ps_qkv, "mm", nb, qk_i)
                    if False:
                        for qk_i, (dst, col0, bias_ap) in enumerate(((Q2, 0, bq), (K2, 128, bk))):
                            ps = ps_qkv.tile([128, 512], F32, tag="mm")
                            ns = slice(nb * 512, (nb + 1) * 512)
                            nc.tensor.matmul(ps, lhsT=_cast(w_qk[0][:, col0:col0 + 128], f32r_mm),
                                             rhs=_cast(h_t[0][:, ns], f32r_mm), start=True, stop=False)
                            nc.tensor.matmul(ps, lhsT=_cast(w_qk[1][:, col0:col0 + 128], f32r_mm),
                                             rhs=_cast(h_t[1][:, ns], f32r_mm), start=False, stop=True)
                            if qk_i == 0:
                                # ACT's free affine port does the bias add
                                # (identity(1.0*x + bias)); splitting the
                                # PSUM evacuations between ACT and DVE stops
                                # the DVE from pacing this phase alone
                                nc.scalar.activation(
                                    dst[:, ns], ps,
                                    mybir.ActivationFunctionType.Identity,
                                    bias=bias_ap, scale=1.0,
                                )
                            else:
                                nc.vector.tensor_scalar(
                                    out=dst[:, ns], in0=ps, scalar1=bias_ap, scalar2=None,
                                    op0=mybir.AluOpType.add,
                                )
                    ones1 = consts.tile([128, 1], F32, tag="ones1")
                    nc.vector.memset(ones1, 1.0)
                    nc.vector.tensor_copy(
                        V1[:, :, :, 64:65].rearrange("p a b o -> p (a b o)"),
                        ones1.to_broadcast((128, MB * 2)),
                    )
                    # V^T chunks are produced just-in-time inside phase C.

                # ---- phase C: attention ----
                A_lo = xh_pool.tile([64, N], MMD, tag="x", name="A_lo")
                A_hi = xh_pool.tile([64, N], MMD, tag="x", name="A_hi")
                with (
                    tc.tile_pool(name="ps_qk", bufs=2, space="PSUM") as ps_qk,
                    tc.tile_pool(name="ps_acc", bufs=4, space="PSUM") as ps_acc,
                ):
                    def emit_proj(nb):
                        ns = slice(nb * 512, (nb + 1) * 512)
                        for m in range(2):
                            ps = ps_acc.tile([128, 512], F32, tag="pv", name=f"pj_{nb}_{m}")
                            nc.tensor.matmul(ps, lhsT=wp_lo[:, m * 128:(m + 1) * 128],
                                             rhs=A_lo[:, ns], start=True, stop=False)
                            nc.tensor.matmul(ps, lhsT=wp_hi[:, m * 128:(m + 1) * 128],
                                             rhs=A_hi[:, ns], start=False, stop=True)
                            y_sb = yout.tile([128, 512], F32, tag="y")
                            nc.vector.tensor_copy(y_sb, ps)
                            if not ablate_dma or (m == 0 and nb == NB - 1):
                                nc.sync.dma_start(out=yp[m * 128:(m + 1) * 128, ns], in_=y_sb)

                    def emit_qk(nb, mb):
                        if ablate2 == "pvonly":
                            return None
                        ns = slice(nb * 512, (nb + 1) * 512)
                        ms_ = slice(mb * 128, (mb + 1) * 128)
                        qk = ps_qk.tile([128, 1024], F32, tag="qk", name=f"qk_{nb}_{mb}")
                        nc.tensor.matmul(qk[:, 0:512], lhsT=_cast(K2[0:64, ms_], f32r_attn),
                                         rhs=_cast(Q2[0:64, ns], f32r_attn), start=True, stop=True,
                                         skip_group_check=True)
                        nc.tensor.matmul(qk[:, 512:1024], lhsT=_cast(K2[64:128, ms_], f32r_attn),
                                         rhs=_cast(Q2[64:128, ns], f32r_attn), start=True, stop=True,
                                         skip_group_check=True)
                        return qk

                    if ablate2 in ("noexp", "pvonly"):
                        pexp_const = consts.tile([128, 1024], PVD, tag="pexpc")
                        nc.vector.tensor_copy(pexp_const,
                                              ones1.to_broadcast((128, 1024)))
                    # software pipeline: emit iteration i+1's QK matmuls before
                    # iteration i's PV matmuls, so the in-order PE queue never
                    # stalls behind a PV that waits on ScalarE's exp.
                    iters = [(nb, mb) for nb in range(NB) for mb in range(MB)]
                    # The PV accumulation for chunk c runs LAG iterations
                    # after its pexp producer was issued (m-chunk order is
                    # commutative), so the exp/Schraudolph producers always
                    # have ~2 iterations of lead time and the in-order PE
                    # queue never waits on them.
                    LAG = 2
                    pv_tiles = {}
                    pend = {}
                    qk_cur = emit_qk(*iters[0])
                    for pos in range(len(iters) + LAG):
                        if pos < len(iters):
                            nb, mb = iters[pos]
                            if ablate2 in ("noexp", "pvonly"):
                                pexp = pexp_const
                            elif mb % 2 == 0:
                                # offload ~25% of the exp work from the
                                # saturated ACT engine to the DVE via the
                                # Schraudolph bit-trick: one f32->int32
                                # affine tensor_scalar, then reinterpret the
                                # high 16 bits as bf16 (~1.5% rel err on
                                # these tiles; blended error stays well
                                # under the 2e-2 gate).
                                pexp_i = pexpp.tile([128, 1024], mybir.dt.int32,
                                                    tag="pexp", name=f"pexpi_{nb}_{mb}")
                                nc.vector.tensor_scalar(
                                    out=pexp_i, in0=qk_cur,
                                    scalar1=SCHRA_A, scalar2=SCHRA_B,
                                    op0=mybir.AluOpType.mult,
                                    op1=mybir.AluOpType.add,
                                )
                                pexp = pexp_i.bitcast(mybir.dt.bfloat16)[:, 1::2]
                            else:
                                pexp = pexpp.tile([128, 1024], PVD, tag="pexp", name=f"pexp_{nb}_{mb}")
                                nc.scalar.activation(pexp, qk_cur,
                                                     mybir.ActivationFunctionType.Exp,
                                                     scale=SCALE)
                            pend[pos] = (pexp, nb, mb)
                            if pos + 1 < len(iters):
                                qk_cur = emit_qk(*iters[pos + 1])
                            if nb == 0:
                                # just-in-time V^T for chunk mb (consumed by
                                # the lagged PV LAG iterations later)
                                psv = ps_acc.tile([128, 128], F32, tag="pv",
                                                  name=f"psv_{mb}")
                                cs = slice(mb * 128, (mb + 1) * 128)
                                nc.tensor.matmul(psv, lhsT=h_t[0][:, cs],
                                                 rhs=w_v[0], start=True, stop=False)
                                nc.tensor.matmul(psv, lhsT=h_t[1][:, cs],
                                                 rhs=w_v[1], start=False, stop=True)
                                if has_v_bias:
                                    nc.vector.tensor_add(
                                        V1[:, mb, :, 0:64],
                                        psv.rearrange("p (h d) -> p h d", h=2),
                                        vb_sb.rearrange("p (h d) -> p h d", h=2),
                                    )
                                elif mb % 2 == 1:
                                    # odd chunks evacuate via the ACT copy
                                    # port to unclog the DVE during nb==0
                                    nc.scalar.activation(
                                        V1[:, mb, :, 0:64],
                                        psv.rearrange("p (h d) -> p h d", h=2),
                                        mybir.ActivationFunctionType.Identity,
                                    )
                                else:
                                    nc.vector.tensor_copy(
                                        V1[:, mb, :, 0:64],
                                        psv.rearrange("p (h d) -> p h d", h=2),
                                    )
                        if pos >= LAG and ablate2 != "nopv":
                            pexp, pnb, pmb = pend.pop(pos - LAG)
                            pns = slice(pnb * 512, (pnb + 1) * 512)
                            if pmb == 0:
                                pv_lo = ps_acc.tile([65, 512], F32, tag="pv", name=f"pvlo_{pnb}")
                                pv_hi = ps_acc.tile([65, 512], F32, tag="pv", name=f"pvhi_{pnb}")
                                pv_tiles[pnb] = (pv_lo, pv_hi)
                            pv_lo, pv_hi = pv_tiles[pnb]
                            nc.tensor.matmul(pv_lo, lhsT=V1[:, pmb, 0, :],
                                             rhs=pexp[:, 0:512],
                                             start=(pmb == 0), stop=(pmb == MB - 1),
                                             skip_group_check=True)
                            nc.tensor.matmul(pv_hi, lhsT=V1[:, pmb, 1, :],
                                             rhs=pexp[:, 512:1024],
                                             start=(pmb == 0), stop=(pmb == MB - 1),
                                             skip_group_check=True)
                            if pmb == MB - 1:
                                # normalize A = pv[0:64] / pv[64].  Release
                                # the PSUM pv slots as fast as possible (two
                                # DVE copies, emitted back-to-back); the
                                # reciprocal + broadcast + divide run later
                                # against the SBUF staging copies so the next
                                # n-block's PV matmuls never stall on them (a
                                # PE stall >3.4us re-throttles the clock).
                                stage = []
                                for pv, A, hx in ((pv_lo, A_lo, 0), (pv_hi, A_hi, 1)):
                                    pvs = recp.tile([65, 512], F32, tag="pvs",
                                                    name=f"pvs_{pnb}_{hx}")
                                    nc.vector.tensor_copy(pvs, pv)
                                    stage.append((pvs, A))
                                for hxd, (pvs, A) in enumerate(stage):
                                    # 1/denominator via exp(-ln(d)) on ACT:
                                    # the DVE's iterative reciprocal is 8
                                    # cycles/element and its approx variant
                                    # miscomputes on HW; ln+exp stay in the
                                    # one loaded activation table set.
                                    rec = recp.tile([1, 512], F32, tag="rec")
                                    nc.scalar.activation(rec, pvs[64:65, :],
                                                         mybir.ActivationFunctionType.Ln,
                                                         scale=1.0)
                                    rb = rbp.tile([64, 512], F32, tag="rb")
                                    if ablate_dma:
                                        nc.vector.memset(rb, 1.0)
                                    else:
                                        # bounce through DRAM: SBUF APs cannot
                                        # have a zero-step partition dim, DRAM
                                        # APs can
                                        dr = drec.tile([1, 512], F32, tag="dr")
                                        nc.sync.dma_start(out=dr, in_=rec)
                                        nc.sync.dma_start(
                                            out=rb,
                                            in_=bass.AP(tensor=dr.tensor,
                                                        offset=dr.offset,
                                                        ap=[[0, 64]] + list(dr.ap[1:])),
                                        )
                                    nc.scalar.activation(rb, rb,
                                                         mybir.ActivationFunctionType.Exp,
                                                         scale=-1.0)
                                    nc.vector.tensor_mul(A[:, pns], pvs[0:64, :], rb)
                            if pmb == 12 and pnb > 0:
                                emit_proj(pnb - 1)

                    if ablate2 != "nopv":
                        # last block's proj (emitted inside the pool scope)
                        emit_proj(NB - 1)
                    else:
                        y_sb = yout.tile([128, 512], F32, tag="y")
                        nc.vector.tensor_copy(y_sb, ones1.to_broadcast((128, 512)))
                        nc.sync.dma_start(out=yp[0:128, 0:512], in_=y_sb)

    nc.finalize()
    return nc


_CACHE = {}


F32R_ATTN = "bf16"
F32R_MM = "bf16"


def _get_program(has_v_bias: bool, chain: int = 1):
    key = ("prog", has_v_bias, F32R_ATTN, F32R_MM, chain)
    if key not in _CACHE:
        _CACHE[key] = _build_program(has_v_bias, F32R_ATTN, F32R_MM, chain)
    return _CACHE[key]


def _make_in_maps(x, gn_w, gn_b, qkv_w, qkv_b, proj_w):
    x = np.ascontiguousarray(x, dtype=np.float32)
    in_maps = []
    for core in range(NCORES):
        b, p = core // 2, core % 2
        rows_q = slice(p * 128, (p + 1) * 128)
        rows_k = slice(256 + p * 128, 256 + (p + 1) * 128)
        rows_v = slice(512 + p * 128, 512 + (p + 1) * 128)
        m = {
            "xb": np.ascontiguousarray(x[b].reshape(C, N)),
            "wqkT": np.ascontiguousarray(
                np.concatenate([qkv_w[rows_q], qkv_w[rows_k]], axis=0).T.astype(np.float32)),
            "wvT": np.ascontiguousarray(qkv_w[rows_v].T.astype(np.float32)),
            "wpT": np.ascontiguousarray(proj_w[:, p * 128:(p + 1) * 128].T.astype(np.float32)),
            "gnw": np.ascontiguousarray(gn_w.astype(np.float32)),
            "gnb": np.ascontiguousarray(gn_b.astype(np.float32)),
            "qkb": np.ascontiguousarray(
                np.concatenate([qkv_b[rows_q], qkv_b[rows_k]]).astype(np.float32)),
        }
        if np.any(qkv_b[512:768]):
            m["vb"] = np.ascontiguousarray(qkv_b[rows_v].astype(np.float32))
        in_maps.append(m)
    return in_maps


def _get_executor(nc, donate=True):
    """Build (once) a cached jitted 8-core executor for the program.

    Mirrors concourse.bass2jax.run_bass_via_pjrt, but caches the jitted
    callable so repeat kernel() calls don't re-trace/re-compile the XLA
    wrapper.  Returns (fn, in_names, out_names) where fn takes a list of
    per-core input dicts and returns a list of per-core output dicts.
    """
    key = ("exec", id(nc), donate)
    if key in _CACHE:
        return _CACHE[key]
    import jax
    import concourse.mybir as _mybir
    from jax.experimental.shard_map import shard_map
    from jax.sharding import Mesh, PartitionSpec
    from concourse import bass2jax

    bass2jax.install_neuronx_cc_hook()
    partition_name = nc.partition_id_tensor.name if nc.partition_id_tensor else None
    in_names, out_names, out_avals, zero_outs = [], [], [], []
    for alloc in nc.m.functions[0].allocations:
        if not isinstance(alloc, _mybir.MemoryLocationSet):
            continue
        name = alloc.memorylocations[0].name
        if alloc.kind == "ExternalInput":
            if name != partition_name:
                in_names.append(name)
        elif alloc.kind == "ExternalOutput":
            shape = tuple(alloc.tensor_shape)
            dtype = _mybir.dt.np(alloc.dtype)
            out_names.append(name)
            out_avals.append(jax.core.ShapedArray(shape, dtype))
            zero_outs.append(np.zeros(shape, dtype))
    n_params = len(in_names)
    n_outs = len(out_avals)
    all_names = in_names + out_names + ([partition_name] if partition_name else [])

    def _body(*args):
        operands = list(args)
        if partition_name is not None:
            operands.append(bass2jax.partition_id_tensor())
        return tuple(bass2jax._bass_exec_p.bind(
            *operands,
            out_avals=tuple(out_avals),
            in_names=tuple(all_names),
            out_names=tuple(out_names),
            lowering_input_output_aliases=(),
            sim_require_finite=True,
            sim_require_nnan=True,
            nc=nc,
        ))

    devices = jax.devices()[:NCORES]
    mesh = Mesh(np.asarray(devices), ("core",))
    in_specs = (PartitionSpec("core"),) * (n_params + n_outs)
    out_specs = (PartitionSpec("core"),) * n_outs
    donate_idx = tuple(range(n_params, n_params + n_outs)) if donate else ()
    sharded = jax.jit(
        shard_map(_body, mesh=mesh, in_specs=in_specs, out_specs=out_specs,
                  check_rep=False),
        donate_argnums=donate_idx, keep_unused=True,
    )

    _CACHE[("sharded", id(nc))] = sharded
    _CACHE[("zeros", id(nc))] = [((NCORES * z.shape[0],) + z.shape[1:], z.dtype)
                                 for z in zero_outs]

    def fn(in_maps):
        concat_in = [
            np.concatenate([np.asarray(in_maps[c][nm]) for c in range(NCORES)], axis=0)
            for nm in in_names
        ]
        concat_zeros = [
            np.zeros((NCORES * z.shape[0], *z.shape[1:]), z.dtype) for z in zero_outs
        ]
        out_arrs = sharded(*concat_in, *concat_zeros)
        return [
            {nm: np.asarray(out_arrs[i]).reshape(NCORES, *out_avals[i].shape)[c]
             for i, nm in enumerate(out_names)}
            for c in range(NCORES)
        ]

    _CACHE[key] = (fn, in_names, out_names)
    return _CACHE[key]


def _prep(inputs):
    x = np.asarray(inputs["x"], dtype=np.float32)
    qkv_b = np.asarray(inputs["qkv_b"], dtype=np.float32)
    has_v_bias = bool(np.any(qkv_b[512:768]))
    nc = _get_program(has_v_bias)
    in_maps = _make_in_maps(
        x,
        np.asarray(inputs["gn_w"], dtype=np.float32),
        np.asarray(inputs["gn_b"], dtype=np.float32),
        np.asarray(inputs["qkv_w"], dtype=np.float32),
        qkv_b,
        np.asarray(inputs["proj_w"], dtype=np.float32),
    )
    return nc, in_maps, x


def run(inputs, trace=False):
    """Run the sharded kernel.  Returns (output, per-core results list)."""
    nc, in_maps, x = _prep(inputs)
    fn, _, _ = _get_executor(nc)
    results = fn(in_maps)
    proj_b = np.asarray(inputs["proj_b"], dtype=np.float32)
    parts = [results[c]["yp"] for c in range(NCORES)]
    y = np.stack([parts[2 * b] + parts[2 * b + 1] for b in range(B)])  # [B, C, N]
    y = y + proj_b[None, :, None]
    out = np.asarray(inputs["x"], dtype=np.float32) + y.reshape(B, C, 64, 64)
    return out.astype(np.float32), results


def _bench_callable(inputs, chain):
    """Device-resident, no-donate timing callable for one chain variant."""
    import jax
    qkv_b = np.asarray(inputs["qkv_b"], dtype=np.float32)
    has_v_bias = bool(np.any(qkv_b[512:768]))
    _, in_maps, _ = _prep(inputs)
    nc = _get_program(has_v_bias, chain=chain)
    key = ("exec", id(nc), False)
    fresh = key not in _CACHE
    _get_executor(nc, donate=False)
    fn, in_names, out_names = _CACHE[key]
    # reach into the jitted callable: rebuild device-resident args
    import concourse.mybir as _mybir
    sharded = _CACHE[("sharded", id(nc))]
    concat_in = _CACHE.get(("dev_in", id(nc)))
    if concat_in is None:
        arrs = []
        for nm in in_names:
            arrs.append(np.concatenate(
                [np.asarray(in_maps[c][nm]) for c in range(NCORES)], axis=0))
        zshapes = _CACHE[("zeros", id(nc))]
        for z in zshapes:
            arrs.append(np.zeros(z[0], z[1]))
        concat_in = [jax.device_put(a) for a in arrs]
        _CACHE[("dev_in", id(nc))] = concat_in

    def call():
        out = sharded(*concat_in)
        jax.block_until_ready(out)

    return call


def bench(inputs, n=25, reps=10):
    """Marginal per-execution device time via an n-times-chained NEFF."""
    import time as _time
    f1 = _bench_callable(inputs, 1)
    fn = _bench_callable(inputs, n)

    def timed(f):
        f()
        best = float("inf")
        for _ in range(reps):
            t0 = _time.perf_counter()
            f()
            best = min(best, _time.perf_counter() - t0)
        return best

    t1, tn = timed(f1), timed(fn)
    return (tn - t1) / (n - 1), t1, tn


def kernel(**inputs) -> np.ndarray:
    out, _ = run(inputs, trace=False)
    return out



# revision 51
# speedup vs baseline: 1.1383x; 1.1383x over previous
"""Trainium2 Bass kernel for nn_AttentionBlock (GroupNorm + 4-head attention + proj + residual).

Sharding: 8 cores = (batch b in 0..3) x (head-pair p in 0..1).  Each core gets
x[b] and the weight slices for its two heads, computes GroupNorm -> QKV ->
attention -> partial proj (over its 128 attention-output channels), and returns
a partial [256, 4096] projection output.  The host sums the two partials per
batch, adds proj bias + residual, and reshapes.

The device program is identical on all cores (SPMD); all per-core variation is
carried by the input tensors.

Layout notes (per core):
  h   = groupnorm(x[b])                SBUF [c=128 x2, n=4096]   (in-place on x)
  Q2  = Wq_pair^T . h                  SBUF [128 (2 heads x 64 d), 4096]
  K2  = Wk_pair^T . h                  SBUF [128, 4096]
  V1  = [h^T . Wv_pair | ones]         SBUF [128 (m-chunk), 32, 2, 65]
  scores^T[m, n] = sum_d K[d,m] Q[d,n] via matmul(lhsT=K2[64 rows], rhs=Q2),
     two heads packed concurrently in PE row-groups (0,0) and (64,0).
  pexp = exp(0.125 * scores^T)         (ScalarE, no max-subtraction)
  pv[d+1, n] += V1[m-chunk]^T . pexp   accumulated over 32 m-chunks in PSUM;
     row 64 (the ones column) accumulates the softmax denominator.
  A = pv[0:64] * (1/pv[64]) broadcast  -> attention output [c, n] per head
  y_partial = Wp[:, pair]^T . A        [256, 4096] -> DRAM
"""

import numpy as np

import concourse.bacc as bacc
import concourse.bass as bass
import concourse.mybir as mybir
import concourse.tile as tile

B = 4
C = 256
N = 4096          # 64*64
NH = 4
D = 64            # head dim
GROUPS = 32
EPS = 1e-5
NCORES = 8
SCALE = float(D) ** -0.5  # 0.125
# scaled by 1/2^16 so the int16-converted result IS the bf16 bit pattern
# of exp(s*SCALE) (bf16 = high half of f32; max reachable pattern ~0x4E40
# stays far below int16 saturation) -> the PV rhs is contiguous bf16 with
# no stride-2 read penalty
SCHRA_A = SCALE * (2.0 ** 23) / float(np.log(2.0)) / 65536.0
SCHRA_B = float(127 * 2 ** 23 - 545947) / 65536.0
F32 = mybir.dt.float32

MB = 32           # m chunks of 128
NB = 8            # n chunks of 512

F32R = mybir.dt.float32r


def _cast(ap, use_f32r):
    """Vestigial: tiles are already declared in the matmul dtype."""
    return ap


def _build_program(has_v_bias: bool, f32r_attn: bool = False, f32r_mm: bool = False, chain: int = 1, ablate_dma: bool = False, ablate2: str = ''):
    nc = bacc.Bacc("TRN2", target_bir_lowering=False)
    MMD = F32R if f32r_mm else F32      # dtype of tiles feeding qkv/proj matmuls
    if f32r_mm == "bf16":
        # f32r runs the qkv/proj matmuls in 2-pass LOW_HIGH mode (~2x the
        # streaming time); bf16 operands run single-pass.
        MMD = mybir.dt.bfloat16
    MAD = F32R if f32r_attn else F32    # dtype of lhsT tiles for QK/PV (K2, V1)
    MADR = MAD                          # dtype of rhs tiles for QK/PV (Q2, pexp)
    if f32r_attn == "bf16":
        MAD = MADR = mybir.dt.bfloat16
    elif f32r_attn == "mixed":
        MAD = F32R
        MADR = mybir.dt.bfloat16
    QKD = MAD   # dtype of Q2/K2 (the QK^T matmul operands)
    PVD = MADR  # dtype of V1/pexp (the PV matmul operands)
    if f32r_attn == "qk_bf16":
        # QK^T in bf16 (fast weight load on the K chunks); PV stays f32r so
        # the ACT exp writes pexp at fp32 speed (bf16 ACT writes measured
        # ~17% slower).  Matmul operand dtypes must match pairwise (walrus
        # verifier), so the split is per-matmul, not per-operand-side.
        QKD = mybir.dt.bfloat16
        PVD = F32R

    xb = nc.dram_tensor("xb", [C, N], F32, kind="ExternalInput")
    wqkT = nc.dram_tensor("wqkT", [C, 256], F32, kind="ExternalInput")   # cols 0:128 Qpair, 128:256 Kpair
    wvT = nc.dram_tensor("wvT", [C, 128], F32, kind="ExternalInput")
    wpT = nc.dram_tensor("wpT", [128, C], F32, kind="ExternalInput")     # rows = pair channels
    gnw = nc.dram_tensor("gnw", [C], F32, kind="ExternalInput")
    gnb = nc.dram_tensor("gnb", [C], F32, kind="ExternalInput")
    qkb = nc.dram_tensor("qkb", [256], F32, kind="ExternalInput")        # 0:128 Q bias pair, 128:256 K bias pair
    if has_v_bias:
        vb = nc.dram_tensor("vb", [128], F32, kind="ExternalInput")
    yp = nc.dram_tensor("yp", [C, N], F32, kind="ExternalOutput")

    # group-indicator matrix: G[c, g] = 1 if c // 8 == g else 0  (per c-tile)
    g_host = np.zeros((128, 16), dtype=np.float32)
    for c in range(128):
        g_host[c, c // 8] = 1.0
    g_dram = nc.inline_tensor(g_host, name="gmat")
    g_dram_t = nc.inline_tensor(np.ascontiguousarray(g_host.T), name="gmatT")

    with tile.TileContext(nc) as tc:
        # Pin the activation table to natural_log_exp_and_others (set 6):
        # every ACT func this kernel uses (Exp, Ln, identity) lives in that
        # one set, so the fixpoint table-load pass should see it loaded on
        # all paths and never re-load (otherwise it thrashes exp<->ln sets
        # at every n-block boundary, ~2.6us each).
        nc.scalar.add_instruction(
            mybir.InstLoadActFuncSet(
                name=nc.get_next_instruction_name(),
                ins=[], outs=[], act_func_set_id=6,
            )
        )
        with (
            tc.tile_pool(name="consts", bufs=1) as consts,
            tc.tile_pool(name="xh", bufs=2) as xh_pool,
            tc.tile_pool(name="hp", bufs=2) as hp_pool,
            tc.tile_pool(name="qk_sb", bufs=1) as qk_sb,
            tc.tile_pool(name="v1p", bufs=1) as v1p,
            tc.tile_pool(name="st", bufs=2) as st,
            tc.tile_pool(name="pexpp", bufs=4) as pexpp,
            tc.tile_pool(name="recp", bufs=4) as recp,
            tc.tile_pool(name="drec", bufs=4, space="DRAM") as drec,
            tc.tile_pool(name="rbp", bufs=4) as rbp,
            tc.tile_pool(name="yout", bufs=4) as yout,
        ):
            # ---- x loads first: everything else gates on them (GroupNorm
            # stats), and each dma_start costs ~0.7us of serial issue time
            # on its trigger queue.  Weight/const DMAs trigger on the idle
            # GPSIMD queue so they don't delay the x chunks.
            x_ts = []
            if chain == 1 and not ablate_dma:
                xtrigs = [nc.sync, nc.sync, nc.sync, nc.sync,
                          nc.scalar, nc.scalar, nc.scalar, nc.scalar]
                for t in range(2):
                    x_t = xh_pool.tile([128, N], F32, tag="x", name=f"x{t}")
                    for q4 in range(4):
                        qs = slice(q4 * (N // 4), (q4 + 1) * (N // 4))
                        xtrigs[t * 4 + q4].dma_start(
                            out=x_t[:, qs], in_=xb[t * 128:(t + 1) * 128, qs])
                    x_ts.append(x_t)

            # ---- constants ----
            g_sb = consts.tile([128, 16], F32, tag="gsb")
            nc.sync.dma_start(out=g_sb, in_=g_dram[:, :])
            g_sbT = consts.tile([16, 128], F32, tag="gsbT")
            nc.scalar.dma_start(out=g_sbT, in_=g_dram_t[:, :])
            def load_weight(shape, dram_slice, tag):
                w_raw = consts.tile(shape, F32, tag=tag + "_r", name=tag + "_r")
                load_weight.q = getattr(load_weight, "q", 0) + 1
                (nc.sync if load_weight.q % 2 else nc.scalar).dma_start(
                    out=w_raw, in_=dram_slice)
                if not f32r_mm:
                    return w_raw
                w_c = consts.tile(shape, MMD, tag=tag, name=tag)
                nc.vector.tensor_copy(w_c, w_raw)
                return w_c

            w_qk = [load_weight([128, 256], wqkT[t * 128:(t + 1) * 128, :], f"wqk{t}")
                    for t in range(2)]
            w_v = [load_weight([128, 128], wvT[t * 128:(t + 1) * 128, :], f"wv{t}")
                   for t in range(2)]
            wp_lo = load_weight([64, 256], wpT[0:64, :], "wplo")
            wp_hi = load_weight([64, 256], wpT[64:128, :], "wphi")

            gw_t, gb_t = [], []
            for t in range(2):
                gwt = consts.tile([128, 1], F32, tag=f"gw{t}")
                nc.gpsimd.dma_start(out=gwt, in_=gnw[t * 128:(t + 1) * 128])
                gw_t.append(gwt)
                gbt = consts.tile([128, 1], F32, tag=f"gb{t}")
                nc.gpsimd.dma_start(out=gbt, in_=gnb[t * 128:(t + 1) * 128])
                gb_t.append(gbt)
            bq = consts.tile([128, 1], F32, tag="bq")
            nc.gpsimd.dma_start(out=bq, in_=qkb[0:128])
            bk = consts.tile([128, 1], F32, tag="bk")
            nc.gpsimd.dma_start(out=bk, in_=qkb[128:256])
            if has_v_bias:
                vb_sb = consts.tile([128, 128], F32, tag="vbsb")
                nc.gpsimd.dma_start(
                    out=vb_sb,
                    in_=bass.AP(tensor=vb, offset=0, ap=[[0, 128], [1, 128]]),
                )
            eps16 = consts.tile([16, 1], F32, tag="eps16")
            nc.vector.memset(eps16, EPS)

            for _rep in range(chain):
                # ---- phase A: GroupNorm (in place: x tile becomes h tile) ----
                h_t = []
                stats_t = []
                scale_t, bias_t = [], []
                with tc.tile_pool(name="ps_g", bufs=2, space="PSUM") as ps_g:
                    for t in range(2):
                        if x_ts:
                            x_t = x_ts[t]
                        else:
                            x_t = xh_pool.tile([128, N], F32, tag="x", name=f"x{t}")
                            if not ablate_dma or _rep == 0:
                                for q4 in range(4):
                                    qs = slice(q4 * (N // 4), (q4 + 1) * (N // 4))
                                    nc.sync.dma_start(out=x_t[:, qs],
                                                      in_=xb[t * 128:(t + 1) * 128, qs])
                            else:
                                nc.vector.memset(x_t[:, 0:1], 0.5)

                        stats = st.tile([128, 8, 6], F32, tag="stats",
                                        name=f"stats{t}")
                        stats_t.append(stats)
                        h_t.append(x_t)
                    # interleaved so tile1's stats don't queue behind all of
                    # tile0's on the in-order DVE
                    for k in range(8):
                        for t in range(2):
                            xr = h_t[t].rearrange("p (k f) -> p k f", f=512)
                            nc.vector.bn_stats(out=stats_t[t][:, k, :],
                                               in_=xr[:, k, :])
                    for t in range(2):
                        x_t = h_t[t]
                        stats = stats_t[t]
                        mv = st.tile([128, 2], F32, tag="mv")
                        nc.vector.bn_aggr(out=mv, in_=stats)

                        # mq = [mean_c, var_c + mean_c^2]
                        mq = st.tile([128, 2], F32, tag="mq")
                        nc.vector.tensor_copy(mq[:, 0:1], mv[:, 0:1])
                        sq = st.tile([128, 1], F32, tag="sq")
                        nc.vector.tensor_mul(sq, mv[:, 0:1], mv[:, 0:1])
                        nc.vector.tensor_add(mq[:, 1:2], mv[:, 1:2], sq)

                        gps = ps_g.tile([16, 2], F32, tag="gps")
                        nc.tensor.matmul(gps, lhsT=g_sb, rhs=mq, start=True, stop=True)
                        # per-group E[x], E[x^2]
                        gsb = st.tile([16, 2], F32, tag="gsb2")
                        nc.scalar.mul(gsb, gps, 0.125)
                        sqg = st.tile([16, 1], F32, tag="sqg")
                        nc.vector.tensor_mul(sqg, gsb[:, 0:1], gsb[:, 0:1])
                        var = st.tile([16, 1], F32, tag="var")
                        nc.vector.tensor_sub(var, gsb[:, 1:2], sqg)
                        lv = st.tile([16, 1], F32, tag="lv")
                        nc.scalar.activation(lv, var, mybir.ActivationFunctionType.Ln,
                                             bias=eps16, scale=1.0)
                        rstd = st.tile([16, 1], F32, tag="rstd")
                        nc.scalar.activation(rstd, lv, mybir.ActivationFunctionType.Exp,
                                             scale=-0.5)
                        ms = st.tile([16, 2], F32, tag=f"ms{t}")
                        nc.vector.tensor_copy(ms[:, 0:1], gsb[:, 0:1])
                        nc.vector.tensor_copy(ms[:, 1:2], rstd)

                        # broadcast per-group stats back to per-channel
                        # [128,2] with a tiny matmul on the (otherwise idle)
                        # PE -- two DMA round-trips were ~3us of critical path
                        bps = ps_g.tile([128, 2], F32, tag="bps")
                        nc.tensor.matmul(bps, lhsT=g_sbT, rhs=ms, start=True,
                                         stop=True)
                        bmean = bps[:, 0:1]
                        brstd = bps[:, 1:2]
                        sc = consts.tile([128, 1], F32, tag=f"scale{t}")
                        nc.vector.tensor_mul(sc, brstd, gw_t[t])
                        scale_t.append(sc)
                        tmp = st.tile([128, 1], F32, tag="tmpb")
                        nc.vector.tensor_mul(tmp, bmean, sc)
                        bi = consts.tile([128, 1], F32, tag=f"bias{t}")
                        nc.vector.tensor_sub(bi, gb_t[t], tmp)
                        bias_t.append(bi)

                    x_t_list = h_t
                    h_t = []
                    for t in range(2):
                        # h = x * scale + bias (chunked: downstream QKV
                        # matmuls depend on whole-tile writes)
                        h_new = hp_pool.tile([128, N], MMD, tag="h", name=f"h{t}")
                        for q4 in range(4):
                            qs = slice(q4 * (N // 4), (q4 + 1) * (N // 4))
                            nc.vector.tensor_scalar(
                                out=h_new[:, qs], in0=x_t_list[t][:, qs],
                                scalar1=scale_t[t], scalar2=bias_t[t],
                                op0=mybir.AluOpType.mult, op1=mybir.AluOpType.add,
                            )
                        h_t.append(h_new)

                # ---- phase B: QKV ----
                Q2 = qk_sb.tile([128, N], QKD, tag="q2")
                K2 = qk_sb.tile([128, N], QKD, tag="k2")
                V1 = v1p.tile([128, MB, 2, 65], PVD, tag="v1")
                def emit_qkv_chunk(pool, tag, cnb, qk_i):
                    dst, col0, bias_ap = ((Q2, 0, bq), (K2, 128, bk))[qk_i]
                    ps = pool.tile([128, 512], F32, tag=tag,
                                   name=f"qkv_{cnb}_{qk_i}")
                    ns = slice(cnb * 512, (cnb + 1) * 512)
                    nc.tensor.matmul(ps, lhsT=_cast(w_qk[0][:, col0:col0 + 128], f32r_mm),
                                     rhs=_cast(h_t[0][:, ns], f32r_mm), start=True, stop=False)
                    nc.tensor.matmul(ps, lhsT=_cast(w_qk[1][:, col0:col0 + 128], f32r_mm),
                                     rhs=_cast(h_t[1][:, ns], f32r_mm), start=False, stop=True)
                    if qk_i == 0:
                        # ACT's free affine port does the bias add
                        nc.scalar.activation(
                            dst[:, ns], ps,
                            mybir.ActivationFunctionType.Identity,
                            bias=bias_ap, scale=1.0,
                        )
                    else:
                        nc.vector.tensor_scalar(
                            out=dst[:, ns], in0=ps, scalar1=bias_ap, scalar2=None,
                            op0=mybir.AluOpType.add,
                        )

                with tc.tile_pool(name="ps_qkv", bufs=3, space="PSUM") as ps_qkv:
                    for nb in range(NB):
                        for qk_i in range(2):
                            emit_qkv_chunk(ps_qkv, "mm", nb, qk_i)
                    if False:
                        for qk_i, (dst, col0, bias_ap) in enumerate(((Q2, 0, bq), (K2, 128, bk))):
                            ps = ps_qkv.tile([128, 512], F32, tag="mm")
                            ns = slice(nb * 512, (nb + 1) * 512)
                            nc.tensor.matmul(ps, lhsT=_cast(w_qk[0][:, col0:col0 + 128], f32r_mm),
                                             rhs=_cast(h_t[0][:, ns], f32r_mm), start=True, stop=False)
                            nc.tensor.matmul(ps, lhsT=_cast(w_qk[1][:, col0:col0 + 128], f32r_mm),
                                             rhs=_cast(h_t[1][:, ns], f32r_mm), start=False, stop=True)
                            if qk_i == 0:
                                # ACT's free affine port does the bias add
                                # (identity(1.0*x + bias)); splitting the
                                # PSUM evacuations between ACT and DVE stops
                                # the DVE from pacing this phase alone
                                nc.scalar.activation(
                                    dst[:, ns], ps,
                                    mybir.ActivationFunctionType.Identity,
                                    bias=bias_ap, scale=1.0,
                                )
                            else:
                                nc.vector.tensor_scalar(
                                    out=dst[:, ns], in0=ps, scalar1=bias_ap, scalar2=None,
                                    op0=mybir.AluOpType.add,
                                )
                    ones1 = consts.tile([128, 1], F32, tag="ones1")
                    nc.vector.memset(ones1, 1.0)
                    nc.vector.tensor_copy(
                        V1[:, :, :, 64:65].rearrange("p a b o -> p (a b o)"),
                        ones1.to_broadcast((128, MB * 2)),
                    )
                    # V^T chunks are produced just-in-time inside phase C.

                # ---- phase C: attention ----
                A_lo = xh_pool.tile([64, N], MMD, tag="x", name="A_lo")
                A_hi = xh_pool.tile([64, N], MMD, tag="x", name="A_hi")
                with (
                    tc.tile_pool(name="ps_qk", bufs=2, space="PSUM") as ps_qk,
                    tc.tile_pool(name="ps_acc", bufs=4, space="PSUM") as ps_acc,
                ):
                    def emit_proj(nb):
                        ns = slice(nb * 512, (nb + 1) * 512)
                        for m in range(2):
                            ps = ps_acc.tile([128, 512], F32, tag="pv", name=f"pj_{nb}_{m}")
                            nc.tensor.matmul(ps, lhsT=wp_lo[:, m * 128:(m + 1) * 128],
                                             rhs=A_lo[:, ns], start=True, stop=False)
                            nc.tensor.matmul(ps, lhsT=wp_hi[:, m * 128:(m + 1) * 128],
                                             rhs=A_hi[:, ns], start=False, stop=True)
                            y_sb = yout.tile([128, 512], F32, tag="y")
                            nc.vector.tensor_copy(y_sb, ps)
                            if not ablate_dma or (m == 0 and nb == NB - 1):
                                nc.sync.dma_start(out=yp[m * 128:(m + 1) * 128, ns], in_=y_sb)

                    def emit_qk(nb, mb):
                        if ablate2 == "pvonly":
                            return None
                        ns = slice(nb * 512, (nb + 1) * 512)
                        ms_ = slice(mb * 128, (mb + 1) * 128)
                        qk = ps_qk.tile([128, 1024], F32, tag="qk", name=f"qk_{nb}_{mb}")
                        nc.tensor.matmul(qk[:, 0:512], lhsT=_cast(K2[0:64, ms_], f32r_attn),
                                         rhs=_cast(Q2[0:64, ns], f32r_attn), start=True, stop=True,
                                         skip_group_check=True)
                        nc.tensor.matmul(qk[:, 512:1024], lhsT=_cast(K2[64:128, ms_], f32r_attn),
                                         rhs=_cast(Q2[64:128, ns], f32r_attn), start=True, stop=True,
                                         skip_group_check=True)
                        return qk

                    if ablate2 in ("noexp", "pvonly"):
                        pexp_const = consts.tile([128, 1024], PVD, tag="pexpc")
                        nc.vector.tensor_copy(pexp_const,
                                              ones1.to_broadcast((128, 1024)))
                    # software pipeline: emit iteration i+1's QK matmuls before
                    # iteration i's PV matmuls, so the in-order PE queue never
                    # stalls behind a PV that waits on ScalarE's exp.
                    iters = [(nb, mb) for nb in range(NB) for mb in range(MB)]
                    # The PV accumulation for chunk c runs LAG iterations
                    # after its pexp producer was issued (m-chunk order is
                    # commutative), so the exp/Schraudolph producers always
                    # have ~2 iterations of lead time and the in-order PE
                    # queue never waits on them.
                    LAG = 2
                    pv_tiles = {}
                    pend = {}
                    qk_cur = emit_qk(*iters[0])
                    for pos in range(len(iters) + LAG):
                        if pos < len(iters):
                            nb, mb = iters[pos]
                            if ablate2 in ("noexp", "pvonly"):
                                pexp = pexp_const
                            elif mb % 2 == 0:
                                # offload ~25% of the exp work from the
                                # saturated ACT engine to the DVE via the
                                # Schraudolph bit-trick: one f32->int32
                                # affine tensor_scalar, then reinterpret the
                                # high 16 bits as bf16 (~1.5% rel err on
                                # these tiles; blended error stays well
                                # under the 2e-2 gate).
                                pexp_i = pexpp.tile([128, 1024], mybir.dt.int16,
                                                    tag="pexp", name=f"pexpi_{nb}_{mb}")
                                nc.vector.tensor_scalar(
                                    out=pexp_i, in0=qk_cur,
                                    scalar1=SCHRA_A, scalar2=SCHRA_B,
                                    op0=mybir.AluOpType.mult,
                                    op1=mybir.AluOpType.add,
                                )
                                pexp = pexp_i.bitcast(mybir.dt.bfloat16)
                            else:
                                pexp = pexpp.tile([128, 1024], PVD, tag="pexp", name=f"pexp_{nb}_{mb}")
                                nc.scalar.activation(pexp, qk_cur,
                                                     mybir.ActivationFunctionType.Exp,
                                                     scale=SCALE)
                            pend[pos] = (pexp, nb, mb)
                            if pos + 1 < len(iters):
                                qk_cur = emit_qk(*iters[pos + 1])
                            if nb == 0:
                                # just-in-time V^T for chunk mb (consumed by
                                # the lagged PV LAG iterations later)
                                psv = ps_acc.tile([128, 128], F32, tag="pv",
                                                  name=f"psv_{mb}")
                                cs = slice(mb * 128, (mb + 1) * 128)
                                nc.tensor.matmul(psv, lhsT=h_t[0][:, cs],
                                                 rhs=w_v[0], start=True, stop=False)
                                nc.tensor.matmul(psv, lhsT=h_t[1][:, cs],
                                                 rhs=w_v[1], start=False, stop=True)
                                if has_v_bias:
                                    nc.vector.tensor_add(
                                        V1[:, mb, :, 0:64],
                                        psv.rearrange("p (h d) -> p h d", h=2),
                                        vb_sb.rearrange("p (h d) -> p h d", h=2),
                                    )
                                elif mb % 2 == 1:
                                    # odd chunks evacuate via the ACT copy
                                    # port to unclog the DVE during nb==0
                                    nc.scalar.activation(
                                        V1[:, mb, :, 0:64],
                                        psv.rearrange("p (h d) -> p h d", h=2),
                                        mybir.ActivationFunctionType.Identity,
                                    )
                                else:
                                    nc.vector.tensor_copy(
                                        V1[:, mb, :, 0:64],
                                        psv.rearrange("p (h d) -> p h d", h=2),
                                    )
                        if pos >= LAG and ablate2 != "nopv":
                            pexp, pnb, pmb = pend.pop(pos - LAG)
                            pns = slice(pnb * 512, (pnb + 1) * 512)
                            if pmb == 0:
                                pv_lo = ps_acc.tile([65, 512], F32, tag="pv", name=f"pvlo_{pnb}")
                                pv_hi = ps_acc.tile([65, 512], F32, tag="pv", name=f"pvhi_{pnb}")
                                pv_tiles[pnb] = (pv_lo, pv_hi)
                            pv_lo, pv_hi = pv_tiles[pnb]
                            nc.tensor.matmul(pv_lo, lhsT=V1[:, pmb, 0, :],
                                             rhs=pexp[:, 0:512],
                                             start=(pmb == 0), stop=(pmb == MB - 1),
                                             skip_group_check=True)
                            nc.tensor.matmul(pv_hi, lhsT=V1[:, pmb, 1, :],
                                             rhs=pexp[:, 512:1024],
                                             start=(pmb == 0), stop=(pmb == MB - 1),
                                             skip_group_check=True)
                            if pmb == MB - 1:
                                # normalize A = pv[0:64] / pv[64].  Release
                                # the PSUM pv slots as fast as possible (two
                                # DVE copies, emitted back-to-back); the
                                # reciprocal + broadcast + divide run later
                                # against the SBUF staging copies so the next
                                # n-block's PV matmuls never stall on them (a
                                # PE stall >3.4us re-throttles the clock).
                                stage = []
                                for pv, A, hx in ((pv_lo, A_lo, 0), (pv_hi, A_hi, 1)):
                                    pvs = recp.tile([65, 512], F32, tag="pvs",
                                                    name=f"pvs_{pnb}_{hx}")
                                    nc.vector.tensor_copy(pvs, pv)
                                    stage.append((pvs, A))
                                for hxd, (pvs, A) in enumerate(stage):
                                    # 1/denominator via exp(-ln(d)) on ACT:
                                    # the DVE's iterative reciprocal is 8
                                    # cycles/element and its approx variant
                                    # miscomputes on HW; ln+exp stay in the
                                    # one loaded activation table set.
                                    rec = recp.tile([1, 512], F32, tag="rec")
                                    nc.scalar.activation(rec, pvs[64:65, :],
                                                         mybir.ActivationFunctionType.Ln,
                                                         scale=1.0)
                                    rb = rbp.tile([64, 512], F32, tag="rb")
                                    if ablate_dma:
                                        nc.vector.memset(rb, 1.0)
                                    else:
                                        # bounce through DRAM: SBUF APs cannot
                                        # have a zero-step partition dim, DRAM
                                        # APs can
                                        dr = drec.tile([1, 512], F32, tag="dr")
                                        nc.sync.dma_start(out=dr, in_=rec)
                                        nc.sync.dma_start(
                                            out=rb,
                                            in_=bass.AP(tensor=dr.tensor,
                                                        offset=dr.offset,
                                                        ap=[[0, 64]] + list(dr.ap[1:])),
                                        )
                                    nc.scalar.activation(rb, rb,
                                                         mybir.ActivationFunctionType.Exp,
                                                         scale=-1.0)
                                    nc.vector.tensor_mul(A[:, pns], pvs[0:64, :], rb)
                            if pmb == 12 and pnb > 0:
                                emit_proj(pnb - 1)

                    if ablate2 != "nopv":
                        # last block's proj (emitted inside the pool scope)
                        emit_proj(NB - 1)
                    else:
                        y_sb = yout.tile([128, 512], F32, tag="y")
                        nc.vector.tensor_copy(y_sb, ones1.to_broadcast((128, 512)))
                        nc.sync.dma_start(out=yp[0:128, 0:512], in_=y_sb)

    nc.finalize()
    return nc


_CACHE = {}


F32R_ATTN = "bf16"
F32R_MM = "bf16"


def _get_program(has_v_bias: bool, chain: int = 1):
    key = ("prog", has_v_bias, F32R_ATTN, F32R_MM, chain)
    if key not in _CACHE:
        _CACHE[key] = _build_program(has_v_bias, F32R_ATTN, F32R_MM, chain)
    return _CACHE[key]


def _make_in_maps(x, gn_w, gn_b, qkv_w, qkv_b, proj_w):
    x = np.ascontiguousarray(x, dtype=np.float32)
    in_maps = []
    for core in range(NCORES):
        b, p = core // 2, core % 2
        rows_q = slice(p * 128, (p + 1) * 128)
        rows_k = slice(256 + p * 128, 256 + (p + 1) * 128)
        rows_v = slice(512 + p * 128, 512 + (p + 1) * 128)
        m = {
            "xb": np.ascontiguousarray(x[b].reshape(C, N)),
            "wqkT": np.ascontiguousarray(
                np.concatenate([qkv_w[rows_q], qkv_w[rows_k]], axis=0).T.astype(np.float32)),
            "wvT": np.ascontiguousarray(qkv_w[rows_v].T.astype(np.float32)),
            "wpT": np.ascontiguousarray(proj_w[:, p * 128:(p + 1) * 128].T.astype(np.float32)),
            "gnw": np.ascontiguousarray(gn_w.astype(np.float32)),
            "gnb": np.ascontiguousarray(gn_b.astype(np.float32)),
            "qkb": np.ascontiguousarray(
                np.concatenate([qkv_b[rows_q], qkv_b[rows_k]]).astype(np.float32)),
        }
        if np.any(qkv_b[512:768]):
            m["vb"] = np.ascontiguousarray(qkv_b[rows_v].astype(np.float32))
        in_maps.append(m)
    return in_maps


def _get_executor(nc, donate=True):
    """Build (once) a cached jitted 8-core executor for the program.

    Mirrors concourse.bass2jax.run_bass_via_pjrt, but caches the jitted
    callable so repeat kernel() calls don't re-trace/re-compile the XLA
    wrapper.  Returns (fn, in_names, out_names) where fn takes a list of
    per-core input dicts and returns a list of per-core output dicts.
    """
    key = ("exec", id(nc), donate)
    if key in _CACHE:
        return _CACHE[key]
    import jax
    import concourse.mybir as _mybir
    from jax.experimental.shard_map import shard_map
    from jax.sharding import Mesh, PartitionSpec
    from concourse import bass2jax

    bass2jax.install_neuronx_cc_hook()
    partition_name = nc.partition_id_tensor.name if nc.partition_id_tensor else None
    in_names, out_names, out_avals, zero_outs = [], [], [], []
    for alloc in nc.m.functions[0].allocations:
        if not isinstance(alloc, _mybir.MemoryLocationSet):
            continue
        name = alloc.memorylocations[0].name
        if alloc.kind == "ExternalInput":
            if name != partition_name:
                in_names.append(name)
        elif alloc.kind == "ExternalOutput":
            shape = tuple(alloc.tensor_shape)
            dtype = _mybir.dt.np(alloc.dtype)
            out_names.append(name)
            out_avals.append(jax.core.ShapedArray(shape, dtype))
            zero_outs.append(np.zeros(shape, dtype))
    n_params = len(in_names)
    n_outs = len(out_avals)
    all_names = in_names + out_names + ([partition_name] if partition_name else [])

    def _body(*args):
        operands = list(args)
        if partition_name is not None:
            operands.append(bass2jax.partition_id_tensor())
        return tuple(bass2jax._bass_exec_p.bind(
            *operands,
            out_avals=tuple(out_avals),
            in_names=tuple(all_names),
            out_names=tuple(out_names),
            lowering_input_output_aliases=(),
            sim_require_finite=True,
            sim_require_nnan=True,
            nc=nc,
        ))

    devices = jax.devices()[:NCORES]
    mesh = Mesh(np.asarray(devices), ("core",))
    in_specs = (PartitionSpec("core"),) * (n_params + n_outs)
    out_specs = (PartitionSpec("core"),) * n_outs
    donate_idx = tuple(range(n_params, n_params + n_outs)) if donate else ()
    sharded = jax.jit(
        shard_map(_body, mesh=mesh, in_specs=in_specs, out_specs=out_specs,
                  check_rep=False),
        donate_argnums=donate_idx, keep_unused=True,
    )

    _CACHE[("sharded", id(nc))] = sharded
    _CACHE[("zeros", id(nc))] = [((NCORES * z.shape[0],) + z.shape[1:], z.dtype)
                                 for z in zero_outs]

    def fn(in_maps):
        concat_in = [
            np.concatenate([np.asarray(in_maps[c][nm]) for c in range(NCORES)], axis=0)
            for nm in in_names
        ]
        concat_zeros = [
            np.zeros((NCORES * z.shape[0], *z.shape[1:]), z.dtype) for z in zero_outs
        ]
        out_arrs = sharded(*concat_in, *concat_zeros)
        return [
            {nm: np.asarray(out_arrs[i]).reshape(NCORES, *out_avals[i].shape)[c]
             for i, nm in enumerate(out_names)}
            for c in range(NCORES)
        ]

    _CACHE[key] = (fn, in_names, out_names)
    return _CACHE[key]


def _prep(inputs):
    x = np.asarray(inputs["x"], dtype=np.float32)
    qkv_b = np.asarray(inputs["qkv_b"], dtype=np.float32)
    has_v_bias = bool(np.any(qkv_b[512:768]))
    nc = _get_program(has_v_bias)
    in_maps = _make_in_maps(
        x,
        np.asarray(inputs["gn_w"], dtype=np.float32),
        np.asarray(inputs["gn_b"], dtype=np.float32),
        np.asarray(inputs["qkv_w"], dtype=np.float32),
        qkv_b,
        np.asarray(inputs["proj_w"], dtype=np.float32),
    )
    return nc, in_maps, x


def run(inputs, trace=False):
    """Run the sharded kernel.  Returns (output, per-core results list)."""
    nc, in_maps, x = _prep(inputs)
    fn, _, _ = _get_executor(nc)
    results = fn(in_maps)
    proj_b = np.asarray(inputs["proj_b"], dtype=np.float32)
    parts = [results[c]["yp"] for c in range(NCORES)]
    y = np.stack([parts[2 * b] + parts[2 * b + 1] for b in range(B)])  # [B, C, N]
    y = y + proj_b[None, :, None]
    out = np.asarray(inputs["x"], dtype=np.float32) + y.reshape(B, C, 64, 64)
    return out.astype(np.float32), results


def _bench_callable(inputs, chain):
    """Device-resident, no-donate timing callable for one chain variant."""
    import jax
    qkv_b = np.asarray(inputs["qkv_b"], dtype=np.float32)
    has_v_bias = bool(np.any(qkv_b[512:768]))
    _, in_maps, _ = _prep(inputs)
    nc = _get_program(has_v_bias, chain=chain)
    key = ("exec", id(nc), False)
    fresh = key not in _CACHE
    _get_executor(nc, donate=False)
    fn, in_names, out_names = _CACHE[key]
    # reach into the jitted callable: rebuild device-resident args
    import concourse.mybir as _mybir
    sharded = _CACHE[("sharded", id(nc))]
    concat_in = _CACHE.get(("dev_in", id(nc)))
    if concat_in is None:
        arrs = []
        for nm in in_names:
            arrs.append(np.concatenate(
                [np.asarray(in_maps[c][nm]) for c in range(NCORES)], axis=0))
        zshapes = _CACHE[("zeros", id(nc))]
        for z in zshapes:
            arrs.append(np.zeros(z[0], z[1]))
        concat_in = [jax.device_put(a) for a in arrs]
        _CACHE[("dev_in", id(nc))] = concat_in

    def call():
        out = sharded(*concat_in)
        jax.block_until_ready(out)

    return call


def bench(inputs, n=25, reps=10):
    """Marginal per-execution device time via an n-times-chained NEFF."""
    import time as _time
    f1 = _bench_callable(inputs, 1)
    fn = _bench_callable(inputs, n)

    def timed(f):
        f()
        best = float("inf")
        for _ in range(reps):
            t0 = _time.perf_counter()
            f()
            best = min(best, _time.perf_counter() - t0)
        return best

    t1, tn = timed(f1), timed(fn)
    return (tn - t1) / (n - 1), t1, tn


def kernel(**inputs) -> np.ndarray:
    out, _ = run(inputs, trace=False)
    return out



# revision 52
# speedup vs baseline: 1.1801x; 1.0368x over previous
"""Trainium2 Bass kernel for nn_AttentionBlock (GroupNorm + 4-head attention + proj + residual).

Sharding: 8 cores = (batch b in 0..3) x (head-pair p in 0..1).  Each core gets
x[b] and the weight slices for its two heads, computes GroupNorm -> QKV ->
attention -> partial proj (over its 128 attention-output channels), and returns
a partial [256, 4096] projection output.  The host sums the two partials per
batch, adds proj bias + residual, and reshapes.

The device program is identical on all cores (SPMD); all per-core variation is
carried by the input tensors.

Layout notes (per core):
  h   = groupnorm(x[b])                SBUF [c=128 x2, n=4096]   (in-place on x)
  Q2  = Wq_pair^T . h                  SBUF [128 (2 heads x 64 d), 4096]
  K2  = Wk_pair^T . h                  SBUF [128, 4096]
  V1  = [h^T . Wv_pair | ones]         SBUF [128 (m-chunk), 32, 2, 65]
  scores^T[m, n] = sum_d K[d,m] Q[d,n] via matmul(lhsT=K2[64 rows], rhs=Q2),
     two heads packed concurrently in PE row-groups (0,0) and (64,0).
  pexp = exp(0.125 * scores^T)         (ScalarE, no max-subtraction)
  pv[d+1, n] += V1[m-chunk]^T . pexp   accumulated over 32 m-chunks in PSUM;
     row 64 (the ones column) accumulates the softmax denominator.
  A = pv[0:64] * (1/pv[64]) broadcast  -> attention output [c, n] per head
  y_partial = Wp[:, pair]^T . A        [256, 4096] -> DRAM
"""

import numpy as np

import concourse.bacc as bacc
import concourse.bass as bass
import concourse.mybir as mybir
import concourse.tile as tile

B = 4
C = 256
N = 4096          # 64*64
NH = 4
D = 64            # head dim
GROUPS = 32
EPS = 1e-5
NCORES = 8
SCALE = float(D) ** -0.5  # 0.125
# scaled by 1/2^16 so the int16-converted result IS the bf16 bit pattern
# of exp(s*SCALE) (bf16 = high half of f32; max reachable pattern ~0x4E40
# stays far below int16 saturation) -> the PV rhs is contiguous bf16 with
# no stride-2 read penalty
SCHRA_A = SCALE * (2.0 ** 23) / float(np.log(2.0)) / 65536.0
SCHRA_B = float(127 * 2 ** 23 - 545947) / 65536.0
F32 = mybir.dt.float32

MB = 32           # m chunks of 128
NB = 8            # n chunks of 512

F32R = mybir.dt.float32r


def _cast(ap, use_f32r):
    """Vestigial: tiles are already declared in the matmul dtype."""
    return ap


def _build_program(has_v_bias: bool, f32r_attn: bool = False, f32r_mm: bool = False, chain: int = 1, ablate_dma: bool = False, ablate2: str = ''):
    nc = bacc.Bacc("TRN2", target_bir_lowering=False)
    MMD = F32R if f32r_mm else F32      # dtype of tiles feeding qkv/proj matmuls
    if f32r_mm == "bf16":
        # f32r runs the qkv/proj matmuls in 2-pass LOW_HIGH mode (~2x the
        # streaming time); bf16 operands run single-pass.
        MMD = mybir.dt.bfloat16
    MAD = F32R if f32r_attn else F32    # dtype of lhsT tiles for QK/PV (K2, V1)
    MADR = MAD                          # dtype of rhs tiles for QK/PV (Q2, pexp)
    if f32r_attn == "bf16":
        MAD = MADR = mybir.dt.bfloat16
    elif f32r_attn == "mixed":
        MAD = F32R
        MADR = mybir.dt.bfloat16
    QKD = MAD   # dtype of Q2/K2 (the QK^T matmul operands)
    PVD = MADR  # dtype of V1/pexp (the PV matmul operands)
    if f32r_attn == "qk_bf16":
        # QK^T in bf16 (fast weight load on the K chunks); PV stays f32r so
        # the ACT exp writes pexp at fp32 speed (bf16 ACT writes measured
        # ~17% slower).  Matmul operand dtypes must match pairwise (walrus
        # verifier), so the split is per-matmul, not per-operand-side.
        QKD = mybir.dt.bfloat16
        PVD = F32R

    xb = nc.dram_tensor("xb", [C, N], F32, kind="ExternalInput")
    wqkT = nc.dram_tensor("wqkT", [C, 256], F32, kind="ExternalInput")   # cols 0:128 Qpair, 128:256 Kpair
    wvT = nc.dram_tensor("wvT", [C, 128], F32, kind="ExternalInput")
    wpT = nc.dram_tensor("wpT", [128, C], F32, kind="ExternalInput")     # rows = pair channels
    gnw = nc.dram_tensor("gnw", [C], F32, kind="ExternalInput")
    gnb = nc.dram_tensor("gnb", [C], F32, kind="ExternalInput")
    qkb = nc.dram_tensor("qkb", [256], F32, kind="ExternalInput")        # 0:128 Q bias pair, 128:256 K bias pair
    if has_v_bias:
        vb = nc.dram_tensor("vb", [128], F32, kind="ExternalInput")
    yp = nc.dram_tensor("yp", [C, N], F32, kind="ExternalOutput")

    # group-indicator matrix: G[c, g] = 1 if c // 8 == g else 0  (per c-tile)
    g_host = np.zeros((128, 16), dtype=np.float32)
    for c in range(128):
        g_host[c, c // 8] = 1.0
    g_dram = nc.inline_tensor(g_host, name="gmat")
    g_dram_t = nc.inline_tensor(np.ascontiguousarray(g_host.T), name="gmatT")

    with tile.TileContext(nc) as tc:
        # Pin the activation table to natural_log_exp_and_others (set 6):
        # every ACT func this kernel uses (Exp, Ln, identity) lives in that
        # one set, so the fixpoint table-load pass should see it loaded on
        # all paths and never re-load (otherwise it thrashes exp<->ln sets
        # at every n-block boundary, ~2.6us each).
        nc.scalar.add_instruction(
            mybir.InstLoadActFuncSet(
                name=nc.get_next_instruction_name(),
                ins=[], outs=[], act_func_set_id=6,
            )
        )
        with (
            tc.tile_pool(name="consts", bufs=1) as consts,
            tc.tile_pool(name="xh", bufs=2) as xh_pool,
            tc.tile_pool(name="hp", bufs=2) as hp_pool,
            tc.tile_pool(name="qk_sb", bufs=1) as qk_sb,
            tc.tile_pool(name="v1p", bufs=1) as v1p,
            tc.tile_pool(name="st", bufs=2) as st,
            tc.tile_pool(name="pexpp", bufs=4) as pexpp,
            tc.tile_pool(name="recp", bufs=4) as recp,
            tc.tile_pool(name="drec", bufs=4, space="DRAM") as drec,
            tc.tile_pool(name="rbp", bufs=4) as rbp,
            tc.tile_pool(name="yout", bufs=4) as yout,
        ):
            # ---- x loads first: everything else gates on them (GroupNorm
            # stats), and each dma_start costs ~0.7us of serial issue time
            # on its trigger queue.  Weight/const DMAs trigger on the idle
            # GPSIMD queue so they don't delay the x chunks.
            x_ts = []
            if chain == 1 and not ablate_dma:
                xtrigs = [nc.sync, nc.sync, nc.sync, nc.sync,
                          nc.scalar, nc.scalar, nc.scalar, nc.scalar]
                for t in range(2):
                    x_t = xh_pool.tile([128, N], F32, tag="x", name=f"x{t}")
                    for q4 in range(4):
                        qs = slice(q4 * (N // 4), (q4 + 1) * (N // 4))
                        xtrigs[t * 4 + q4].dma_start(
                            out=x_t[:, qs], in_=xb[t * 128:(t + 1) * 128, qs])
                    x_ts.append(x_t)

            # ---- constants ----
            g_sb = consts.tile([128, 16], F32, tag="gsb")
            nc.sync.dma_start(out=g_sb, in_=g_dram[:, :])
            g_sbT = consts.tile([16, 128], F32, tag="gsbT")
            nc.scalar.dma_start(out=g_sbT, in_=g_dram_t[:, :])
            def load_weight(shape, dram_slice, tag):
                w_raw = consts.tile(shape, F32, tag=tag + "_r", name=tag + "_r")
                load_weight.q = getattr(load_weight, "q", 0) + 1
                (nc.sync if load_weight.q % 2 else nc.scalar).dma_start(
                    out=w_raw, in_=dram_slice)
                if not f32r_mm:
                    return w_raw
                w_c = consts.tile(shape, MMD, tag=tag, name=tag)
                nc.vector.tensor_copy(w_c, w_raw)
                return w_c

            w_qk = [load_weight([128, 256], wqkT[t * 128:(t + 1) * 128, :], f"wqk{t}")
                    for t in range(2)]
            w_v = [load_weight([128, 128], wvT[t * 128:(t + 1) * 128, :], f"wv{t}")
                   for t in range(2)]
            wp_lo = load_weight([64, 256], wpT[0:64, :], "wplo")
            wp_hi = load_weight([64, 256], wpT[64:128, :], "wphi")

            gw_t, gb_t = [], []
            for t in range(2):
                gwt = consts.tile([128, 1], F32, tag=f"gw{t}")
                nc.gpsimd.dma_start(out=gwt, in_=gnw[t * 128:(t + 1) * 128])
                gw_t.append(gwt)
                gbt = consts.tile([128, 1], F32, tag=f"gb{t}")
                nc.gpsimd.dma_start(out=gbt, in_=gnb[t * 128:(t + 1) * 128])
                gb_t.append(gbt)
            bq = consts.tile([128, 1], F32, tag="bq")
            nc.gpsimd.dma_start(out=bq, in_=qkb[0:128])
            bk = consts.tile([128, 1], F32, tag="bk")
            nc.gpsimd.dma_start(out=bk, in_=qkb[128:256])
            if has_v_bias:
                vb_sb = consts.tile([128, 128], F32, tag="vbsb")
                nc.gpsimd.dma_start(
                    out=vb_sb,
                    in_=bass.AP(tensor=vb, offset=0, ap=[[0, 128], [1, 128]]),
                )
            eps16 = consts.tile([16, 1], F32, tag="eps16")
            nc.vector.memset(eps16, EPS)

            for _rep in range(chain):
                # ---- phase A: GroupNorm (in place: x tile becomes h tile) ----
                h_t = []
                stats_t = []
                scale_t, bias_t = [], []
                with tc.tile_pool(name="ps_g", bufs=2, space="PSUM") as ps_g:
                    for t in range(2):
                        if x_ts:
                            x_t = x_ts[t]
                        else:
                            x_t = xh_pool.tile([128, N], F32, tag="x", name=f"x{t}")
                            if not ablate_dma or _rep == 0:
                                for q4 in range(4):
                                    qs = slice(q4 * (N // 4), (q4 + 1) * (N // 4))
                                    nc.sync.dma_start(out=x_t[:, qs],
                                                      in_=xb[t * 128:(t + 1) * 128, qs])
                            else:
                                nc.vector.memset(x_t[:, 0:1], 0.5)

                        stats = st.tile([128, 8, 6], F32, tag="stats",
                                        name=f"stats{t}")
                        stats_t.append(stats)
                        h_t.append(x_t)
                    # interleaved so tile1's stats don't queue behind all of
                    # tile0's on the in-order DVE
                    for k in range(8):
                        for t in range(2):
                            xr = h_t[t].rearrange("p (k f) -> p k f", f=512)
                            nc.vector.bn_stats(out=stats_t[t][:, k, :],
                                               in_=xr[:, k, :])
                    for t in range(2):
                        x_t = h_t[t]
                        stats = stats_t[t]
                        mv = st.tile([128, 2], F32, tag="mv")
                        nc.vector.bn_aggr(out=mv, in_=stats)

                        # mq = [mean_c, var_c + mean_c^2]
                        mq = st.tile([128, 2], F32, tag="mq")
                        nc.vector.tensor_copy(mq[:, 0:1], mv[:, 0:1])
                        sq = st.tile([128, 1], F32, tag="sq")
                        nc.vector.tensor_mul(sq, mv[:, 0:1], mv[:, 0:1])
                        nc.vector.tensor_add(mq[:, 1:2], mv[:, 1:2], sq)

                        gps = ps_g.tile([16, 2], F32, tag="gps")
                        nc.tensor.matmul(gps, lhsT=g_sb, rhs=mq, start=True, stop=True)
                        # per-group E[x], E[x^2]
                        gsb = st.tile([16, 2], F32, tag="gsb2")
                        nc.scalar.mul(gsb, gps, 0.125)
                        sqg = st.tile([16, 1], F32, tag="sqg")
                        nc.vector.tensor_mul(sqg, gsb[:, 0:1], gsb[:, 0:1])
                        var = st.tile([16, 1], F32, tag="var")
                        nc.vector.tensor_sub(var, gsb[:, 1:2], sqg)
                        lv = st.tile([16, 1], F32, tag="lv")
                        nc.scalar.activation(lv, var, mybir.ActivationFunctionType.Ln,
                                             bias=eps16, scale=1.0)
                        rstd = st.tile([16, 1], F32, tag="rstd")
                        nc.scalar.activation(rstd, lv, mybir.ActivationFunctionType.Exp,
                                             scale=-0.5)
                        ms = st.tile([16, 2], F32, tag=f"ms{t}")
                        nc.vector.tensor_copy(ms[:, 0:1], gsb[:, 0:1])
                        nc.vector.tensor_copy(ms[:, 1:2], rstd)

                        # broadcast per-group stats back to per-channel
                        # [128,2] with a tiny matmul on the (otherwise idle)
                        # PE -- two DMA round-trips were ~3us of critical path
                        bps = ps_g.tile([128, 2], F32, tag="bps")
                        nc.tensor.matmul(bps, lhsT=g_sbT, rhs=ms, start=True,
                                         stop=True)
                        bmean = bps[:, 0:1]
                        brstd = bps[:, 1:2]
                        sc = consts.tile([128, 1], F32, tag=f"scale{t}")
                        nc.vector.tensor_mul(sc, brstd, gw_t[t])
                        scale_t.append(sc)
                        tmp = st.tile([128, 1], F32, tag="tmpb")
                        nc.vector.tensor_mul(tmp, bmean, sc)
                        bi = consts.tile([128, 1], F32, tag=f"bias{t}")
                        nc.vector.tensor_sub(bi, gb_t[t], tmp)
                        bias_t.append(bi)

                    x_t_list = h_t
                    h_t = []
                    for t in range(2):
                        # h = x * scale + bias (chunked: downstream QKV
                        # matmuls depend on whole-tile writes)
                        h_new = hp_pool.tile([128, N], MMD, tag="h", name=f"h{t}")
                        for q4 in range(4):
                            qs = slice(q4 * (N // 4), (q4 + 1) * (N // 4))
                            if q4 % 2 == 1:
                                # ACT's affine port (idle in the prologue;
                                # the DVE is the prologue pacer)
                                nc.scalar.activation(
                                    h_new[:, qs], x_t_list[t][:, qs],
                                    mybir.ActivationFunctionType.Identity,
                                    bias=bias_t[t], scale=scale_t[t],
                                )
                            else:
                                nc.vector.tensor_scalar(
                                    out=h_new[:, qs], in0=x_t_list[t][:, qs],
                                    scalar1=scale_t[t], scalar2=bias_t[t],
                                    op0=mybir.AluOpType.mult, op1=mybir.AluOpType.add,
                                )
                        h_t.append(h_new)

                # ---- phase B: QKV ----
                Q2 = qk_sb.tile([128, N], QKD, tag="q2")
                K2 = qk_sb.tile([128, N], QKD, tag="k2")
                V1 = v1p.tile([128, MB, 2, 65], PVD, tag="v1")
                def emit_qkv_chunk(pool, tag, cnb, qk_i):
                    dst, col0, bias_ap = ((Q2, 0, bq), (K2, 128, bk))[qk_i]
                    ps = pool.tile([128, 512], F32, tag=tag,
                                   name=f"qkv_{cnb}_{qk_i}")
                    ns = slice(cnb * 512, (cnb + 1) * 512)
                    nc.tensor.matmul(ps, lhsT=_cast(w_qk[0][:, col0:col0 + 128], f32r_mm),
                                     rhs=_cast(h_t[0][:, ns], f32r_mm), start=True, stop=False)
                    nc.tensor.matmul(ps, lhsT=_cast(w_qk[1][:, col0:col0 + 128], f32r_mm),
                                     rhs=_cast(h_t[1][:, ns], f32r_mm), start=False, stop=True)
                    if qk_i == 0:
                        # ACT's free affine port does the bias add
                        nc.scalar.activation(
                            dst[:, ns], ps,
                            mybir.ActivationFunctionType.Identity,
                            bias=bias_ap, scale=1.0,
                        )
                    else:
                        nc.vector.tensor_scalar(
                            out=dst[:, ns], in0=ps, scalar1=bias_ap, scalar2=None,
                            op0=mybir.AluOpType.add,
                        )

                with tc.tile_pool(name="ps_qkv", bufs=3, space="PSUM") as ps_qkv:
                    for nb in range(NB):
                        for qk_i in range(2):
                            emit_qkv_chunk(ps_qkv, "mm", nb, qk_i)
                    if False:
                        for qk_i, (dst, col0, bias_ap) in enumerate(((Q2, 0, bq), (K2, 128, bk))):
                            ps = ps_qkv.tile([128, 512], F32, tag="mm")
                            ns = slice(nb * 512, (nb + 1) * 512)
                            nc.tensor.matmul(ps, lhsT=_cast(w_qk[0][:, col0:col0 + 128], f32r_mm),
                                             rhs=_cast(h_t[0][:, ns], f32r_mm), start=True, stop=False)
                            nc.tensor.matmul(ps, lhsT=_cast(w_qk[1][:, col0:col0 + 128], f32r_mm),
                                             rhs=_cast(h_t[1][:, ns], f32r_mm), start=False, stop=True)
                            if qk_i == 0:
                                # ACT's free affine port does the bias add
                                # (identity(1.0*x + bias)); splitting the
                                # PSUM evacuations between ACT and DVE stops
                                # the DVE from pacing this phase alone
                                nc.scalar.activation(
                                    dst[:, ns], ps,
                                    mybir.ActivationFunctionType.Identity,
                                    bias=bias_ap, scale=1.0,
                                )
                            else:
                                nc.vector.tensor_scalar(
                                    out=dst[:, ns], in0=ps, scalar1=bias_ap, scalar2=None,
                                    op0=mybir.AluOpType.add,
                                )
                    ones1 = consts.tile([128, 1], F32, tag="ones1")
                    nc.vector.memset(ones1, 1.0)
                    nc.vector.tensor_copy(
                        V1[:, :, :, 64:65].rearrange("p a b o -> p (a b o)"),
                        ones1.to_broadcast((128, MB * 2)),
                    )
                    # V^T chunks are produced just-in-time inside phase C.

                # ---- phase C: attention ----
                A_lo = xh_pool.tile([64, N], MMD, tag="x", name="A_lo")
                A_hi = xh_pool.tile([64, N], MMD, tag="x", name="A_hi")
                with (
                    tc.tile_pool(name="ps_qk", bufs=2, space="PSUM") as ps_qk,
                    tc.tile_pool(name="ps_acc", bufs=4, space="PSUM") as ps_acc,
                ):
                    def emit_proj(nb, last=False):
                        ns = slice(nb * 512, (nb + 1) * 512)
                        for m in range(2):
                            ps = ps_acc.tile([128, 512], F32, tag="pv", name=f"pj_{nb}_{m}")
                            nc.tensor.matmul(ps, lhsT=wp_lo[:, m * 128:(m + 1) * 128],
                                             rhs=A_lo[:, ns], start=True, stop=False)
                            nc.tensor.matmul(ps, lhsT=wp_hi[:, m * 128:(m + 1) * 128],
                                             rhs=A_hi[:, ns], start=False, stop=True)
                            y_sb = yout.tile([128, 512], F32, tag="y")
                            if last:
                                nc.scalar.activation(
                                    y_sb, ps,
                                    mybir.ActivationFunctionType.Identity)
                            else:
                                nc.vector.tensor_copy(y_sb, ps)
                            if not ablate_dma or (m == 0 and nb == NB - 1):
                                nc.sync.dma_start(out=yp[m * 128:(m + 1) * 128, ns], in_=y_sb)

                    def emit_qk(nb, mb):
                        if ablate2 == "pvonly":
                            return None
                        ns = slice(nb * 512, (nb + 1) * 512)
                        ms_ = slice(mb * 128, (mb + 1) * 128)
                        qk = ps_qk.tile([128, 1024], F32, tag="qk", name=f"qk_{nb}_{mb}")
                        nc.tensor.matmul(qk[:, 0:512], lhsT=_cast(K2[0:64, ms_], f32r_attn),
                                         rhs=_cast(Q2[0:64, ns], f32r_attn), start=True, stop=True,
                                         skip_group_check=True)
                        nc.tensor.matmul(qk[:, 512:1024], lhsT=_cast(K2[64:128, ms_], f32r_attn),
                                         rhs=_cast(Q2[64:128, ns], f32r_attn), start=True, stop=True,
                                         skip_group_check=True)
                        return qk

                    if ablate2 in ("noexp", "pvonly"):
                        pexp_const = consts.tile([128, 1024], PVD, tag="pexpc")
                        nc.vector.tensor_copy(pexp_const,
                                              ones1.to_broadcast((128, 1024)))
                    # software pipeline: emit iteration i+1's QK matmuls before
                    # iteration i's PV matmuls, so the in-order PE queue never
                    # stalls behind a PV that waits on ScalarE's exp.
                    iters = [(nb, mb) for nb in range(NB) for mb in range(MB)]
                    # The PV accumulation for chunk c runs LAG iterations
                    # after its pexp producer was issued (m-chunk order is
                    # commutative), so the exp/Schraudolph producers always
                    # have ~2 iterations of lead time and the in-order PE
                    # queue never waits on them.
                    LAG = 2
                    pv_tiles = {}
                    pend = {}
                    qk_cur = emit_qk(*iters[0])
                    for pos in range(len(iters) + LAG):
                        if pos < len(iters):
                            nb, mb = iters[pos]
                            if ablate2 in ("noexp", "pvonly"):
                                pexp = pexp_const
                            elif mb % 2 == 0:
                                # offload ~25% of the exp work from the
                                # saturated ACT engine to the DVE via the
                                # Schraudolph bit-trick: one f32->int32
                                # affine tensor_scalar, then reinterpret the
                                # high 16 bits as bf16 (~1.5% rel err on
                                # these tiles; blended error stays well
                                # under the 2e-2 gate).
                                pexp_i = pexpp.tile([128, 1024], mybir.dt.int16,
                                                    tag="pexp", name=f"pexpi_{nb}_{mb}")
                                nc.vector.tensor_scalar(
                                    out=pexp_i, in0=qk_cur,
                                    scalar1=SCHRA_A, scalar2=SCHRA_B,
                                    op0=mybir.AluOpType.mult,
                                    op1=mybir.AluOpType.add,
                                )
                                pexp = pexp_i.bitcast(mybir.dt.bfloat16)
                            else:
                                pexp = pexpp.tile([128, 1024], PVD, tag="pexp", name=f"pexp_{nb}_{mb}")
                                nc.scalar.activation(pexp, qk_cur,
                                                     mybir.ActivationFunctionType.Exp,
                                                     scale=SCALE)
                            pend[pos] = (pexp, nb, mb)
                            if pos + 1 < len(iters):
                                qk_cur = emit_qk(*iters[pos + 1])
                            if nb == 0:
                                # just-in-time V^T for chunk mb (consumed by
                                # the lagged PV LAG iterations later)
                                psv = ps_acc.tile([128, 128], F32, tag="pv",
                                                  name=f"psv_{mb}")
                                cs = slice(mb * 128, (mb + 1) * 128)
                                nc.tensor.matmul(psv, lhsT=h_t[0][:, cs],
                                                 rhs=w_v[0], start=True, stop=False)
                                nc.tensor.matmul(psv, lhsT=h_t[1][:, cs],
                                                 rhs=w_v[1], start=False, stop=True)
                                if has_v_bias:
                                    nc.vector.tensor_add(
                                        V1[:, mb, :, 0:64],
                                        psv.rearrange("p (h d) -> p h d", h=2),
                                        vb_sb.rearrange("p (h d) -> p h d", h=2),
                                    )
                                elif mb % 2 == 1:
                                    # odd chunks evacuate via the ACT copy
                                    # port to unclog the DVE during nb==0
                                    nc.scalar.activation(
                                        V1[:, mb, :, 0:64],
                                        psv.rearrange("p (h d) -> p h d", h=2),
                                        mybir.ActivationFunctionType.Identity,
                                    )
                                else:
                                    nc.vector.tensor_copy(
                                        V1[:, mb, :, 0:64],
                                        psv.rearrange("p (h d) -> p h d", h=2),
                                    )
                        if pos >= LAG and ablate2 != "nopv":
                            pexp, pnb, pmb = pend.pop(pos - LAG)
                            pns = slice(pnb * 512, (pnb + 1) * 512)
                            if pmb == 0:
                                pv_lo = ps_acc.tile([65, 512], F32, tag="pv", name=f"pvlo_{pnb}")
                                pv_hi = ps_acc.tile([65, 512], F32, tag="pv", name=f"pvhi_{pnb}")
                                pv_tiles[pnb] = (pv_lo, pv_hi)
                            pv_lo, pv_hi = pv_tiles[pnb]
                            nc.tensor.matmul(pv_lo, lhsT=V1[:, pmb, 0, :],
                                             rhs=pexp[:, 0:512],
                                             start=(pmb == 0), stop=(pmb == MB - 1),
                                             skip_group_check=True)
                            nc.tensor.matmul(pv_hi, lhsT=V1[:, pmb, 1, :],
                                             rhs=pexp[:, 512:1024],
                                             start=(pmb == 0), stop=(pmb == MB - 1),
                                             skip_group_check=True)
                            if pmb == MB - 1:
                                # normalize A = pv[0:64] / pv[64].  Release
                                # the PSUM pv slots as fast as possible (two
                                # DVE copies, emitted back-to-back); the
                                # reciprocal + broadcast + divide run later
                                # against the SBUF staging copies so the next
                                # n-block's PV matmuls never stall on them (a
                                # PE stall >3.4us re-throttles the clock).
                                stage = []
                                for pv, A, hx in ((pv_lo, A_lo, 0), (pv_hi, A_hi, 1)):
                                    pvs = recp.tile([65, 512], F32, tag="pvs",
                                                    name=f"pvs_{pnb}_{hx}")
                                    nc.vector.tensor_copy(pvs, pv)
                                    stage.append((pvs, A))
                                for hxd, (pvs, A) in enumerate(stage):
                                    # 1/denominator via exp(-ln(d)) on ACT:
                                    # the DVE's iterative reciprocal is 8
                                    # cycles/element and its approx variant
                                    # miscomputes on HW; ln+exp stay in the
                                    # one loaded activation table set.
                                    rec = recp.tile([1, 512], F32, tag="rec")
                                    nc.scalar.activation(rec, pvs[64:65, :],
                                                         mybir.ActivationFunctionType.Ln,
                                                         scale=1.0)
                                    rb = rbp.tile([64, 512], F32, tag="rb")
                                    if ablate_dma:
                                        nc.vector.memset(rb, 1.0)
                                    elif pnb == NB - 1:
                                        nc.gpsimd.partition_broadcast(rb, rec)
                                    else:
                                        # bounce through DRAM: SBUF APs cannot
                                        # have a zero-step partition dim, DRAM
                                        # APs can
                                        dr = drec.tile([1, 512], F32, tag="dr")
                                        nc.sync.dma_start(out=dr, in_=rec)
                                        nc.sync.dma_start(
                                            out=rb,
                                            in_=bass.AP(tensor=dr.tensor,
                                                        offset=dr.offset,
                                                        ap=[[0, 64]] + list(dr.ap[1:])),
                                        )
                                    nc.scalar.activation(rb, rb,
                                                         mybir.ActivationFunctionType.Exp,
                                                         scale=-1.0)
                                    nc.vector.tensor_mul(A[:, pns], pvs[0:64, :], rb)
                            if pmb == 12 and pnb > 0:
                                emit_proj(pnb - 1)

                    if ablate2 != "nopv":
                        # last block's proj (emitted inside the pool scope)
                        emit_proj(NB - 1, last=True)
                    else:
                        y_sb = yout.tile([128, 512], F32, tag="y")
                        nc.vector.tensor_copy(y_sb, ones1.to_broadcast((128, 512)))
                        nc.sync.dma_start(out=yp[0:128, 0:512], in_=y_sb)

    nc.finalize()
    return nc


_CACHE = {}


F32R_ATTN = "bf16"
F32R_MM = "bf16"


def _get_program(has_v_bias: bool, chain: int = 1):
    key = ("prog", has_v_bias, F32R_ATTN, F32R_MM, chain)
    if key not in _CACHE:
        _CACHE[key] = _build_program(has_v_bias, F32R_ATTN, F32R_MM, chain)
    return _CACHE[key]


def _make_in_maps(x, gn_w, gn_b, qkv_w, qkv_b, proj_w):
    x = np.ascontiguousarray(x, dtype=np.float32)
    in_maps = []
    for core in range(NCORES):
        b, p = core // 2, core % 2
        rows_q = slice(p * 128, (p + 1) * 128)
        rows_k = slice(256 + p * 128, 256 + (p + 1) * 128)
        rows_v = slice(512 + p * 128, 512 + (p + 1) * 128)
        m = {
            "xb": np.ascontiguousarray(x[b].reshape(C, N)),
            "wqkT": np.ascontiguousarray(
                np.concatenate([qkv_w[rows_q], qkv_w[rows_k]], axis=0).T.astype(np.float32)),
            "wvT": np.ascontiguousarray(qkv_w[rows_v].T.astype(np.float32)),
            "wpT": np.ascontiguousarray(proj_w[:, p * 128:(p + 1) * 128].T.astype(np.float32)),
            "gnw": np.ascontiguousarray(gn_w.astype(np.float32)),
            "gnb": np.ascontiguousarray(gn_b.astype(np.float32)),
            "qkb": np.ascontiguousarray(
                np.concatenate([qkv_b[rows_q], qkv_b[rows_k]]).astype(np.float32)),
        }
        if np.any(qkv_b[512:768]):
            m["vb"] = np.ascontiguousarray(qkv_b[rows_v].astype(np.float32))
        in_maps.append(m)
    return in_maps


def _get_executor(nc, donate=True):
    """Build (once) a cached jitted 8-core executor for the program.

    Mirrors concourse.bass2jax.run_bass_via_pjrt, but caches the jitted
    callable so repeat kernel() calls don't re-trace/re-compile the XLA
    wrapper.  Returns (fn, in_names, out_names) where fn takes a list of
    per-core input dicts and returns a list of per-core output dicts.
    """
    key = ("exec", id(nc), donate)
    if key in _CACHE:
        return _CACHE[key]
    import jax
    import concourse.mybir as _mybir
    from jax.experimental.shard_map import shard_map
    from jax.sharding import Mesh, PartitionSpec
    from concourse import bass2jax

    bass2jax.install_neuronx_cc_hook()
    partition_name = nc.partition_id_tensor.name if nc.partition_id_tensor else None
    in_names, out_names, out_avals, zero_outs = [], [], [], []
    for alloc in nc.m.functions[0].allocations:
        if not isinstance(alloc, _mybir.MemoryLocationSet):
            continue
        name = alloc.memorylocations[0].name
        if alloc.kind == "ExternalInput":
            if name != partition_name:
                in_names.append(name)
        elif alloc.kind == "ExternalOutput":
            shape = tuple(alloc.tensor_shape)
            dtype = _mybir.dt.np(alloc.dtype)
            out_names.append(name)
            out_avals.append(jax.core.ShapedArray(shape, dtype))
            zero_outs.append(np.zeros(shape, dtype))
    n_params = len(in_names)
    n_outs = len(out_avals)
    all_names = in_names + out_names + ([partition_name] if partition_name else [])

    def _body(*args):
        operands = list(args)
        if partition_name is not None:
            operands.append(bass2jax.partition_id_tensor())
        return tuple(bass2jax._bass_exec_p.bind(
            *operands,
            out_avals=tuple(out_avals),
            in_names=tuple(all_names),
            out_names=tuple(out_names),
            lowering_input_output_aliases=(),
            sim_require_finite=True,
            sim_require_nnan=True,
            nc=nc,
        ))

    devices = jax.devices()[:NCORES]
    mesh = Mesh(np.asarray(devices), ("core",))
    in_specs = (PartitionSpec("core"),) * (n_params + n_outs)
    out_specs = (PartitionSpec("core"),) * n_outs
    donate_idx = tuple(range(n_params, n_params + n_outs)) if donate else ()
    sharded = jax.jit(
        shard_map(_body, mesh=mesh, in_specs=in_specs, out_specs=out_specs,
                  check_rep=False),
        donate_argnums=donate_idx, keep_unused=True,
    )

    _CACHE[("sharded", id(nc))] = sharded
    _CACHE[("zeros", id(nc))] = [((NCORES * z.shape[0],) + z.shape[1:], z.dtype)
                                 for z in zero_outs]

    def fn(in_maps):
        concat_in = [
            np.concatenate([np.asarray(in_maps[c][nm]) for c in range(NCORES)], axis=0)
            for nm in in_names
        ]
        concat_zeros = [
            np.zeros((NCORES * z.shape[0], *z.shape[1:]), z.dtype) for z in zero_outs
        ]
        out_arrs = sharded(*concat_in, *concat_zeros)
        return [
            {nm: np.asarray(out_arrs[i]).reshape(NCORES, *out_avals[i].shape)[c]
             for i, nm in enumerate(out_names)}
            for c in range(NCORES)
        ]

    _CACHE[key] = (fn, in_names, out_names)
    return _CACHE[key]


def _prep(inputs):
    x = np.asarray(inputs["x"], dtype=np.float32)
    qkv_b = np.asarray(inputs["qkv_b"], dtype=np.float32)
    has_v_bias = bool(np.any(qkv_b[512:768]))
    nc = _get_program(has_v_bias)
    in_maps = _make_in_maps(
        x,
        np.asarray(inputs["gn_w"], dtype=np.float32),
        np.asarray(inputs["gn_b"], dtype=np.float32),
        np.asarray(inputs["qkv_w"], dtype=np.float32),
        qkv_b,
        np.asarray(inputs["proj_w"], dtype=np.float32),
    )
    return nc, in_maps, x


def run(inputs, trace=False):
    """Run the sharded kernel.  Returns (output, per-core results list)."""
    nc, in_maps, x = _prep(inputs)
    fn, _, _ = _get_executor(nc)
    results = fn(in_maps)
    proj_b = np.asarray(inputs["proj_b"], dtype=np.float32)
    parts = [results[c]["yp"] for c in range(NCORES)]
    y = np.stack([parts[2 * b] + parts[2 * b + 1] for b in range(B)])  # [B, C, N]
    y = y + proj_b[None, :, None]
    out = np.asarray(inputs["x"], dtype=np.float32) + y.reshape(B, C, 64, 64)
    return out.astype(np.float32), results


def _bench_callable(inputs, chain):
    """Device-resident, no-donate timing callable for one chain variant."""
    import jax
    qkv_b = np.asarray(inputs["qkv_b"], dtype=np.float32)
    has_v_bias = bool(np.any(qkv_b[512:768]))
    _, in_maps, _ = _prep(inputs)
    nc = _get_program(has_v_bias, chain=chain)
    key = ("exec", id(nc), False)
    fresh = key not in _CACHE
    _get_executor(nc, donate=False)
    fn, in_names, out_names = _CACHE[key]
    # reach into the jitted callable: rebuild device-resident args
    import concourse.mybir as _mybir
    sharded = _CACHE[("sharded", id(nc))]
    concat_in = _CACHE.get(("dev_in", id(nc)))
    if concat_in is None:
        arrs = []
        for nm in in_names:
            arrs.append(np.concatenate(
                [np.asarray(in_maps[c][nm]) for c in range(NCORES)], axis=0))
        zshapes = _CACHE[("zeros", id(nc))]
        for z in zshapes:
            arrs.append(np.zeros(z[0], z[1]))
        concat_in = [jax.device_put(a) for a in arrs]
        _CACHE[("dev_in", id(nc))] = concat_in

    def call():
        out = sharded(*concat_in)
        jax.block_until_ready(out)

    return call


def bench(inputs, n=25, reps=10):
    """Marginal per-execution device time via an n-times-chained NEFF."""
    import time as _time
    f1 = _bench_callable(inputs, 1)
    fn = _bench_callable(inputs, n)

    def timed(f):
        f()
        best = float("inf")
        for _ in range(reps):
            t0 = _time.perf_counter()
            f()
            best = min(best, _time.perf_counter() - t0)
        return best

    t1, tn = timed(f1), timed(fn)
    return (tn - t1) / (n - 1), t1, tn


def kernel(**inputs) -> np.ndarray:
    out, _ = run(inputs, trace=False)
    return out



# revision 58
# speedup vs baseline: 1.1802x; 1.0001x over previous
"""Trainium2 Bass kernel for nn_AttentionBlock (GroupNorm + 4-head attention + proj + residual).

Sharding: 8 cores = (batch b in 0..3) x (head-pair p in 0..1).  Each core gets
x[b] and the weight slices for its two heads, computes GroupNorm -> QKV ->
attention -> partial proj (over its 128 attention-output channels), and returns
a partial [256, 4096] projection output.  The host sums the two partials per
batch, adds proj bias + residual, and reshapes.

The device program is identical on all cores (SPMD); all per-core variation is
carried by the input tensors.

Layout notes (per core):
  h   = groupnorm(x[b])                SBUF [c=128 x2, n=4096]   (in-place on x)
  Q2  = Wq_pair^T . h                  SBUF [128 (2 heads x 64 d), 4096]
  K2  = Wk_pair^T . h                  SBUF [128, 4096]
  V1  = [h^T . Wv_pair | ones]         SBUF [128 (m-chunk), 32, 2, 65]
  scores^T[m, n] = sum_d K[d,m] Q[d,n] via matmul(lhsT=K2[64 rows], rhs=Q2),
     two heads packed concurrently in PE row-groups (0,0) and (64,0).
  pexp = exp(0.125 * scores^T)         (ScalarE, no max-subtraction)
  pv[d+1, n] += V1[m-chunk]^T . pexp   accumulated over 32 m-chunks in PSUM;
     row 64 (the ones column) accumulates the softmax denominator.
  A = pv[0:64] * (1/pv[64]) broadcast  -> attention output [c, n] per head
  y_partial = Wp[:, pair]^T . A        [256, 4096] -> DRAM
"""

import numpy as np

import concourse.bacc as bacc
import concourse.bass as bass
import concourse.mybir as mybir
import concourse.tile as tile

B = 4
C = 256
N = 4096          # 64*64
NH = 4
D = 64            # head dim
GROUPS = 32
EPS = 1e-5
NCORES = 8
SCALE = float(D) ** -0.5  # 0.125
# scaled by 1/2^16 so the int16-converted result IS the bf16 bit pattern
# of exp(s*SCALE) (bf16 = high half of f32; max reachable pattern ~0x4E40
# stays far below int16 saturation) -> the PV rhs is contiguous bf16 with
# no stride-2 read penalty
SCHRA_A = SCALE * (2.0 ** 23) / float(np.log(2.0)) / 65536.0
SCHRA_B = float(127 * 2 ** 23 - 545947) / 65536.0
F32 = mybir.dt.float32

MB = 32           # m chunks of 128
NB = 8            # n chunks of 512

F32R = mybir.dt.float32r


def _cast(ap, use_f32r):
    """Vestigial: tiles are already declared in the matmul dtype."""
    return ap


def _build_program(has_v_bias: bool, f32r_attn: bool = False, f32r_mm: bool = False, chain: int = 1, ablate_dma: bool = False, ablate2: str = ''):
    nc = bacc.Bacc("TRN2", target_bir_lowering=False)
    MMD = F32R if f32r_mm else F32      # dtype of tiles feeding qkv/proj matmuls
    if f32r_mm == "bf16":
        # f32r runs the qkv/proj matmuls in 2-pass LOW_HIGH mode (~2x the
        # streaming time); bf16 operands run single-pass.
        MMD = mybir.dt.bfloat16
    MAD = F32R if f32r_attn else F32    # dtype of lhsT tiles for QK/PV (K2, V1)
    MADR = MAD                          # dtype of rhs tiles for QK/PV (Q2, pexp)
    if f32r_attn == "bf16":
        MAD = MADR = mybir.dt.bfloat16
    elif f32r_attn == "mixed":
        MAD = F32R
        MADR = mybir.dt.bfloat16
    QKD = MAD   # dtype of Q2/K2 (the QK^T matmul operands)
    PVD = MADR  # dtype of V1/pexp (the PV matmul operands)
    if f32r_attn == "qk_bf16":
        # QK^T in bf16 (fast weight load on the K chunks); PV stays f32r so
        # the ACT exp writes pexp at fp32 speed (bf16 ACT writes measured
        # ~17% slower).  Matmul operand dtypes must match pairwise (walrus
        # verifier), so the split is per-matmul, not per-operand-side.
        QKD = mybir.dt.bfloat16
        PVD = F32R

    xb = nc.dram_tensor("xb", [C, N], F32, kind="ExternalInput")
    wqkT = nc.dram_tensor("wqkT", [C, 256], F32, kind="ExternalInput")   # cols 0:128 Qpair, 128:256 Kpair
    wvT = nc.dram_tensor("wvT", [C, 128], F32, kind="ExternalInput")
    wpT = nc.dram_tensor("wpT", [128, C], F32, kind="ExternalInput")     # rows = pair channels
    gnw = nc.dram_tensor("gnw", [C], F32, kind="ExternalInput")
    gnb = nc.dram_tensor("gnb", [C], F32, kind="ExternalInput")
    qkb = nc.dram_tensor("qkb", [256], F32, kind="ExternalInput")        # 0:128 Q bias pair, 128:256 K bias pair
    if has_v_bias:
        vb = nc.dram_tensor("vb", [128], F32, kind="ExternalInput")
    yp = nc.dram_tensor("yp", [C, N], F32, kind="ExternalOutput")

    # group-indicator matrix: G[c, g] = 1 if c // 8 == g else 0  (per c-tile)
    g_host = np.zeros((128, 16), dtype=np.float32)
    for c in range(128):
        g_host[c, c // 8] = 1.0
    g_dram = nc.inline_tensor(g_host, name="gmat")
    g_dram_t = nc.inline_tensor(np.ascontiguousarray(g_host.T), name="gmatT")

    with tile.TileContext(nc) as tc:
        # Pin the activation table to natural_log_exp_and_others (set 6):
        # every ACT func this kernel uses (Exp, Ln, identity) lives in that
        # one set, so the fixpoint table-load pass should see it loaded on
        # all paths and never re-load (otherwise it thrashes exp<->ln sets
        # at every n-block boundary, ~2.6us each).
        nc.scalar.add_instruction(
            mybir.InstLoadActFuncSet(
                name=nc.get_next_instruction_name(),
                ins=[], outs=[], act_func_set_id=6,
            )
        )
        with (
            tc.tile_pool(name="consts", bufs=1) as consts,
            tc.tile_pool(name="xh", bufs=2) as xh_pool,
            tc.tile_pool(name="hp", bufs=2) as hp_pool,
            tc.tile_pool(name="qk_sb", bufs=1) as qk_sb,
            tc.tile_pool(name="v1p", bufs=1) as v1p,
            tc.tile_pool(name="st", bufs=2) as st,
            tc.tile_pool(name="pexpp", bufs=4) as pexpp,
            tc.tile_pool(name="recp", bufs=4) as recp,
            tc.tile_pool(name="drec", bufs=4, space="DRAM") as drec,
            tc.tile_pool(name="rbp", bufs=4) as rbp,
            tc.tile_pool(name="yout", bufs=4) as yout,
        ):
            # ---- x loads first: everything else gates on them (GroupNorm
            # stats), and each dma_start costs ~0.7us of serial issue time
            # on its trigger queue.  Weight/const DMAs trigger on the idle
            # GPSIMD queue so they don't delay the x chunks.
            x_ts = []
            if chain == 1 and not ablate_dma:
                xtrigs = [nc.sync, nc.sync, nc.sync, nc.sync,
                          nc.scalar, nc.scalar, nc.scalar, nc.scalar]
                for t in range(2):
                    x_t = xh_pool.tile([128, N], F32, tag="x", name=f"x{t}")
                    for q4 in range(4):
                        qs = slice(q4 * (N // 4), (q4 + 1) * (N // 4))
                        xtrigs[t * 4 + q4].dma_start(
                            out=x_t[:, qs], in_=xb[t * 128:(t + 1) * 128, qs])
                    x_ts.append(x_t)

            # ---- constants ----
            g_sb = consts.tile([128, 16], F32, tag="gsb")
            nc.sync.dma_start(out=g_sb, in_=g_dram[:, :])
            g_sbT = consts.tile([16, 128], F32, tag="gsbT")
            nc.scalar.dma_start(out=g_sbT, in_=g_dram_t[:, :])
            def load_weight(shape, dram_slice, tag):
                w_raw = consts.tile(shape, F32, tag=tag + "_r", name=tag + "_r")
                load_weight.q = getattr(load_weight, "q", 0) + 1
                (nc.sync if load_weight.q % 2 else nc.scalar).dma_start(
                    out=w_raw, in_=dram_slice)
                if not f32r_mm:
                    return w_raw
                w_c = consts.tile(shape, MMD, tag=tag, name=tag)
                nc.vector.tensor_copy(w_c, w_raw)
                return w_c

            w_qk = [load_weight([128, 256], wqkT[t * 128:(t + 1) * 128, :], f"wqk{t}")
                    for t in range(2)]
            w_v = [load_weight([128, 128], wvT[t * 128:(t + 1) * 128, :], f"wv{t}")
                   for t in range(2)]
            wp_lo = load_weight([64, 256], wpT[0:64, :], "wplo")
            wp_hi = load_weight([64, 256], wpT[64:128, :], "wphi")

            gw_t, gb_t = [], []
            for t in range(2):
                gwt = consts.tile([128, 1], F32, tag=f"gw{t}")
                nc.gpsimd.dma_start(out=gwt, in_=gnw[t * 128:(t + 1) * 128])
                gw_t.append(gwt)
                gbt = consts.tile([128, 1], F32, tag=f"gb{t}")
                nc.gpsimd.dma_start(out=gbt, in_=gnb[t * 128:(t + 1) * 128])
                gb_t.append(gbt)
            bq = consts.tile([128, 1], F32, tag="bq")
            nc.gpsimd.dma_start(out=bq, in_=qkb[0:128])
            bk = consts.tile([128, 1], F32, tag="bk")
            nc.gpsimd.dma_start(out=bk, in_=qkb[128:256])
            if has_v_bias:
                vb_sb = consts.tile([128, 128], F32, tag="vbsb")
                nc.gpsimd.dma_start(
                    out=vb_sb,
                    in_=bass.AP(tensor=vb, offset=0, ap=[[0, 128], [1, 128]]),
                )
            eps16 = consts.tile([16, 1], F32, tag="eps16")
            nc.vector.memset(eps16, EPS)

            for _rep in range(chain):
                # ---- phase A: GroupNorm (in place: x tile becomes h tile) ----
                h_t = []
                stats_t = []
                scale_t, bias_t = [], []
                with tc.tile_pool(name="ps_g", bufs=2, space="PSUM") as ps_g:
                    for t in range(2):
                        if x_ts:
                            x_t = x_ts[t]
                        else:
                            x_t = xh_pool.tile([128, N], F32, tag="x", name=f"x{t}")
                            if not ablate_dma or _rep == 0:
                                for q4 in range(4):
                                    qs = slice(q4 * (N // 4), (q4 + 1) * (N // 4))
                                    nc.sync.dma_start(out=x_t[:, qs],
                                                      in_=xb[t * 128:(t + 1) * 128, qs])
                            else:
                                nc.vector.memset(x_t[:, 0:1], 0.5)

                        stats = st.tile([128, 8, 6], F32, tag="stats",
                                        name=f"stats{t}")
                        stats_t.append(stats)
                        h_t.append(x_t)
                    # interleaved so tile1's stats don't queue behind all of
                    # tile0's on the in-order DVE
                    for k in range(8):
                        for t in range(2):
                            xr = h_t[t].rearrange("p (k f) -> p k f", f=512)
                            nc.vector.bn_stats(out=stats_t[t][:, k, :],
                                               in_=xr[:, k, :])
                    for t in range(2):
                        x_t = h_t[t]
                        stats = stats_t[t]
                        mv = st.tile([128, 2], F32, tag="mv")
                        nc.vector.bn_aggr(out=mv, in_=stats)

                        # mq = [mean_c, var_c + mean_c^2]
                        mq = st.tile([128, 2], F32, tag="mq")
                        nc.vector.tensor_copy(mq[:, 0:1], mv[:, 0:1])
                        sq = st.tile([128, 1], F32, tag="sq")
                        nc.vector.tensor_mul(sq, mv[:, 0:1], mv[:, 0:1])
                        nc.vector.tensor_add(mq[:, 1:2], mv[:, 1:2], sq)

                        gps = ps_g.tile([16, 2], F32, tag="gps")
                        nc.tensor.matmul(gps, lhsT=g_sb, rhs=mq, start=True, stop=True)
                        # per-group E[x], E[x^2]
                        gsb = st.tile([16, 2], F32, tag="gsb2")
                        nc.scalar.mul(gsb, gps, 0.125)
                        sqg = st.tile([16, 1], F32, tag="sqg")
                        nc.vector.tensor_mul(sqg, gsb[:, 0:1], gsb[:, 0:1])
                        var = st.tile([16, 1], F32, tag="var")
                        nc.vector.tensor_sub(var, gsb[:, 1:2], sqg)
                        lv = st.tile([16, 1], F32, tag="lv")
                        nc.scalar.activation(lv, var, mybir.ActivationFunctionType.Ln,
                                             bias=eps16, scale=1.0)
                        rstd = st.tile([16, 1], F32, tag="rstd")
                        nc.scalar.activation(rstd, lv, mybir.ActivationFunctionType.Exp,
                                             scale=-0.5)
                        ms = st.tile([16, 2], F32, tag=f"ms{t}")
                        nc.vector.tensor_copy(ms[:, 0:1], gsb[:, 0:1])
                        nc.vector.tensor_copy(ms[:, 1:2], rstd)

                        # broadcast per-group stats back to per-channel
                        # [128,2] with a tiny matmul on the (otherwise idle)
                        # PE -- two DMA round-trips were ~3us of critical path
                        bps = ps_g.tile([128, 2], F32, tag="bps")
                        nc.tensor.matmul(bps, lhsT=g_sbT, rhs=ms, start=True,
                                         stop=True)
                        bmean = bps[:, 0:1]
                        brstd = bps[:, 1:2]
                        sc = consts.tile([128, 1], F32, tag=f"scale{t}")
                        nc.vector.tensor_mul(sc, brstd, gw_t[t])
                        scale_t.append(sc)
                        tmp = st.tile([128, 1], F32, tag="tmpb")
                        nc.vector.tensor_mul(tmp, bmean, sc)
                        bi = consts.tile([128, 1], F32, tag=f"bias{t}")
                        nc.vector.tensor_sub(bi, gb_t[t], tmp)
                        bias_t.append(bi)

                    x_t_list = h_t
                    h_t = []
                    for t in range(2):
                        # h = x * scale + bias (chunked: downstream QKV
                        # matmuls depend on whole-tile writes)
                        h_new = hp_pool.tile([128, N], MMD, tag="h", name=f"h{t}")
                        for q4 in range(4):
                            qs = slice(q4 * (N // 4), (q4 + 1) * (N // 4))
                            if q4 % 2 == 1:
                                # ACT's affine port (idle in the prologue;
                                # the DVE is the prologue pacer)
                                nc.scalar.activation(
                                    h_new[:, qs], x_t_list[t][:, qs],
                                    mybir.ActivationFunctionType.Identity,
                                    bias=bias_t[t], scale=scale_t[t],
                                )
                            else:
                                nc.vector.tensor_scalar(
                                    out=h_new[:, qs], in0=x_t_list[t][:, qs],
                                    scalar1=scale_t[t], scalar2=bias_t[t],
                                    op0=mybir.AluOpType.mult, op1=mybir.AluOpType.add,
                                )
                        h_t.append(h_new)

                # ---- phase B: QKV ----
                Q2 = qk_sb.tile([128, N], QKD, tag="q2")
                K2 = qk_sb.tile([128, N], QKD, tag="k2")
                V1 = v1p.tile([128, MB, 2, 65], PVD, tag="v1")
                def emit_qkv_chunk(pool, tag, cnb, qk_i):
                    dst, col0, bias_ap = ((Q2, 0, bq), (K2, 128, bk))[qk_i]
                    ps = pool.tile([128, 512], F32, tag=tag,
                                   name=f"qkv_{cnb}_{qk_i}")
                    ns = slice(cnb * 512, (cnb + 1) * 512)
                    nc.tensor.matmul(ps, lhsT=_cast(w_qk[0][:, col0:col0 + 128], f32r_mm),
                                     rhs=_cast(h_t[0][:, ns], f32r_mm), start=True, stop=False)
                    nc.tensor.matmul(ps, lhsT=_cast(w_qk[1][:, col0:col0 + 128], f32r_mm),
                                     rhs=_cast(h_t[1][:, ns], f32r_mm), start=False, stop=True)
                    if qk_i == 0:
                        # ACT's free affine port does the bias add
                        nc.scalar.activation(
                            dst[:, ns], ps,
                            mybir.ActivationFunctionType.Identity,
                            bias=bias_ap, scale=1.0,
                        )
                    else:
                        nc.vector.tensor_scalar(
                            out=dst[:, ns], in0=ps, scalar1=bias_ap, scalar2=None,
                            op0=mybir.AluOpType.add,
                        )

                with tc.tile_pool(name="ps_qkv", bufs=3, space="PSUM") as ps_qkv:
                    for nb in range(NB):
                        for qk_i in range(2):
                            emit_qkv_chunk(ps_qkv, "mm", nb, qk_i)
                    if False:
                        for qk_i, (dst, col0, bias_ap) in enumerate(((Q2, 0, bq), (K2, 128, bk))):
                            ps = ps_qkv.tile([128, 512], F32, tag="mm")
                            ns = slice(nb * 512, (nb + 1) * 512)
                            nc.tensor.matmul(ps, lhsT=_cast(w_qk[0][:, col0:col0 + 128], f32r_mm),
                                             rhs=_cast(h_t[0][:, ns], f32r_mm), start=True, stop=False)
                            nc.tensor.matmul(ps, lhsT=_cast(w_qk[1][:, col0:col0 + 128], f32r_mm),
                                             rhs=_cast(h_t[1][:, ns], f32r_mm), start=False, stop=True)
                            if qk_i == 0:
                                # ACT's free affine port does the bias add
                                # (identity(1.0*x + bias)); splitting the
                                # PSUM evacuations between ACT and DVE stops
                                # the DVE from pacing this phase alone
                                nc.scalar.activation(
                                    dst[:, ns], ps,
                                    mybir.ActivationFunctionType.Identity,
                                    bias=bias_ap, scale=1.0,
                                )
                            else:
                                nc.vector.tensor_scalar(
                                    out=dst[:, ns], in0=ps, scalar1=bias_ap, scalar2=None,
                                    op0=mybir.AluOpType.add,
                                )
                    ones1 = consts.tile([128, 1], F32, tag="ones1")
                    nc.vector.memset(ones1, 1.0)
                    nc.vector.tensor_copy(
                        V1[:, :, :, 64:65].rearrange("p a b o -> p (a b o)"),
                        ones1.to_broadcast((128, MB * 2)),
                    )
                    # V^T chunks are produced just-in-time inside phase C.

                # ---- phase C: attention ----
                A_lo = xh_pool.tile([64, N], MMD, tag="x", name="A_lo")
                A_hi = xh_pool.tile([64, N], MMD, tag="x", name="A_hi")
                with (
                    tc.tile_pool(name="ps_qk", bufs=2, space="PSUM") as ps_qk,
                    tc.tile_pool(name="ps_acc", bufs=4, space="PSUM") as ps_acc,
                ):
                    def emit_proj(nb, last=False):
                        ns = slice(nb * 512, (nb + 1) * 512)
                        for m in range(2):
                            ps = ps_acc.tile([128, 512], F32, tag="pv", name=f"pj_{nb}_{m}")
                            nc.tensor.matmul(ps, lhsT=wp_lo[:, m * 128:(m + 1) * 128],
                                             rhs=A_lo[:, ns], start=True, stop=False)
                            nc.tensor.matmul(ps, lhsT=wp_hi[:, m * 128:(m + 1) * 128],
                                             rhs=A_hi[:, ns], start=False, stop=True)
                            y_sb = yout.tile([128, 512], F32, tag="y")
                            if last:
                                nc.scalar.activation(
                                    y_sb, ps,
                                    mybir.ActivationFunctionType.Identity)
                            else:
                                nc.vector.tensor_copy(y_sb, ps)
                            if not ablate_dma or (m == 0 and nb == NB - 1):
                                nc.sync.dma_start(out=yp[m * 128:(m + 1) * 128, ns], in_=y_sb)

                    def emit_qk(nb, mb):
                        if ablate2 == "pvonly":
                            return None
                        ns = slice(nb * 512, (nb + 1) * 512)
                        ms_ = slice(mb * 128, (mb + 1) * 128)
                        qk = ps_qk.tile([128, 1024], F32, tag="qk", name=f"qk_{nb}_{mb}")
                        nc.tensor.matmul(qk[:, 0:512], lhsT=_cast(K2[0:64, ms_], f32r_attn),
                                         rhs=_cast(Q2[0:64, ns], f32r_attn), start=True, stop=True,
                                         skip_group_check=True)
                        nc.tensor.matmul(qk[:, 512:1024], lhsT=_cast(K2[64:128, ms_], f32r_attn),
                                         rhs=_cast(Q2[64:128, ns], f32r_attn), start=True, stop=True,
                                         skip_group_check=True)
                        return qk

                    if ablate2 in ("noexp", "pvonly"):
                        pexp_const = consts.tile([128, 1024], PVD, tag="pexpc")
                        nc.vector.tensor_copy(pexp_const,
                                              ones1.to_broadcast((128, 1024)))
                    # software pipeline: emit iteration i+1's QK matmuls before
                    # iteration i's PV matmuls, so the in-order PE queue never
                    # stalls behind a PV that waits on ScalarE's exp.
                    iters = [(nb, mb) for nb in range(NB) for mb in range(MB)]
                    # The PV accumulation for chunk c runs LAG iterations
                    # after its pexp producer was issued (m-chunk order is
                    # commutative), so the exp/Schraudolph producers always
                    # have ~2 iterations of lead time and the in-order PE
                    # queue never waits on them.
                    LAG = 2
                    pv_tiles = {}
                    pend = {}
                    qk_cur = emit_qk(*iters[0])
                    for pos in range(len(iters) + LAG):
                        if pos < len(iters):
                            nb, mb = iters[pos]
                            if ablate2 in ("noexp", "pvonly"):
                                pexp = pexp_const
                            elif mb % 2 == 0:
                                # offload ~25% of the exp work from the
                                # saturated ACT engine to the DVE via the
                                # Schraudolph bit-trick: one f32->int32
                                # affine tensor_scalar, then reinterpret the
                                # high 16 bits as bf16 (~1.5% rel err on
                                # these tiles; blended error stays well
                                # under the 2e-2 gate).
                                pexp_i = pexpp.tile([128, 1024], mybir.dt.int16,
                                                    tag="pexp", name=f"pexpi_{nb}_{mb}")
                                nc.vector.tensor_scalar(
                                    out=pexp_i, in0=qk_cur,
                                    scalar1=SCHRA_A, scalar2=SCHRA_B,
                                    op0=mybir.AluOpType.mult,
                                    op1=mybir.AluOpType.add,
                                )
                                pexp = pexp_i.bitcast(mybir.dt.bfloat16)
                            else:
                                pexp = pexpp.tile([128, 1024], PVD, tag="pexp", name=f"pexp_{nb}_{mb}")
                                nc.scalar.activation(pexp, qk_cur,
                                                     mybir.ActivationFunctionType.Exp,
                                                     scale=SCALE)
                            pend[pos] = (pexp, nb, mb)
                            if pos + 1 < len(iters):
                                qk_cur = emit_qk(*iters[pos + 1])
                            if nb == 0:
                                # just-in-time V^T for chunk mb (consumed by
                                # the lagged PV LAG iterations later)
                                psv = ps_acc.tile([128, 128], F32, tag="pv",
                                                  name=f"psv_{mb}")
                                cs = slice(mb * 128, (mb + 1) * 128)
                                nc.tensor.matmul(psv, lhsT=h_t[0][:, cs],
                                                 rhs=w_v[0], start=True, stop=False)
                                nc.tensor.matmul(psv, lhsT=h_t[1][:, cs],
                                                 rhs=w_v[1], start=False, stop=True)
                                if has_v_bias:
                                    nc.vector.tensor_add(
                                        V1[:, mb, :, 0:64],
                                        psv.rearrange("p (h d) -> p h d", h=2),
                                        vb_sb.rearrange("p (h d) -> p h d", h=2),
                                    )
                                elif mb % 2 == 1:
                                    # odd chunks evacuate via the ACT copy
                                    # port to unclog the DVE during nb==0
                                    nc.scalar.activation(
                                        V1[:, mb, :, 0:64],
                                        psv.rearrange("p (h d) -> p h d", h=2),
                                        mybir.ActivationFunctionType.Identity,
                                    )
                                else:
                                    nc.vector.tensor_copy(
                                        V1[:, mb, :, 0:64],
                                        psv.rearrange("p (h d) -> p h d", h=2),
                                    )
                        if pos >= LAG and ablate2 != "nopv":
                            pexp, pnb, pmb = pend.pop(pos - LAG)
                            pns = slice(pnb * 512, (pnb + 1) * 512)
                            if pmb == 0:
                                pv_lo = ps_acc.tile([65, 512], F32, tag="pv", name=f"pvlo_{pnb}")
                                pv_hi = ps_acc.tile([65, 512], F32, tag="pv", name=f"pvhi_{pnb}")
                                pv_tiles[pnb] = (pv_lo, pv_hi)
                            pv_lo, pv_hi = pv_tiles[pnb]
                            nc.tensor.matmul(pv_lo, lhsT=V1[:, pmb, 0, :],
                                             rhs=pexp[:, 0:512],
                                             start=(pmb == 0), stop=(pmb == MB - 1),
                                             skip_group_check=True)
                            nc.tensor.matmul(pv_hi, lhsT=V1[:, pmb, 1, :],
                                             rhs=pexp[:, 512:1024],
                                             start=(pmb == 0), stop=(pmb == MB - 1),
                                             skip_group_check=True)
                            if pmb == MB - 1:
                                # normalize A = pv[0:64] / pv[64].  Release
                                # the PSUM pv slots as fast as possible (two
                                # DVE copies, emitted back-to-back); the
                                # reciprocal + broadcast + divide run later
                                # against the SBUF staging copies so the next
                                # n-block's PV matmuls never stall on them (a
                                # PE stall >3.4us re-throttles the clock).
                                stage = []
                                for pv, A, hx in ((pv_lo, A_lo, 0), (pv_hi, A_hi, 1)):
                                    pvs = recp.tile([65, 512], F32, tag="pvs",
                                                    name=f"pvs_{pnb}_{hx}")
                                    nc.vector.tensor_copy(pvs, pv)
                                    stage.append((pvs, A))
                                for hxd, (pvs, A) in enumerate(stage):
                                    # 1/denominator via exp(-ln(d)) on ACT:
                                    # the DVE's iterative reciprocal is 8
                                    # cycles/element and its approx variant
                                    # miscomputes on HW; ln+exp stay in the
                                    # one loaded activation table set.
                                    rec = recp.tile([1, 512], F32, tag="rec")
                                    nc.scalar.activation(rec, pvs[64:65, :],
                                                         mybir.ActivationFunctionType.Ln,
                                                         scale=1.0)
                                    rb = rbp.tile([64, 512], F32, tag="rb")
                                    if ablate_dma:
                                        nc.vector.memset(rb, 1.0)
                                    elif pnb == NB - 1:
                                        nc.gpsimd.partition_broadcast(rb, rec)
                                    else:
                                        # bounce through DRAM: SBUF APs cannot
                                        # have a zero-step partition dim, DRAM
                                        # APs can
                                        dr = drec.tile([1, 512], F32, tag="dr")
                                        nc.sync.dma_start(out=dr, in_=rec)
                                        nc.sync.dma_start(
                                            out=rb,
                                            in_=bass.AP(tensor=dr.tensor,
                                                        offset=dr.offset,
                                                        ap=[[0, 64]] + list(dr.ap[1:])),
                                        )
                                    nc.scalar.activation(rb, rb,
                                                         mybir.ActivationFunctionType.Exp,
                                                         scale=-1.0)
                                    nc.vector.tensor_mul(A[:, pns], pvs[0:64, :], rb)
                            if pmb == 12 and pnb > 0:
                                emit_proj(pnb - 1)

                    if ablate2 != "nopv":
                        # last block's proj (emitted inside the pool scope)
                        emit_proj(NB - 1, last=True)
                    else:
                        y_sb = yout.tile([128, 512], F32, tag="y")
                        nc.vector.tensor_copy(y_sb, ones1.to_broadcast((128, 512)))
                        nc.sync.dma_start(out=yp[0:128, 0:512], in_=y_sb)

    nc.finalize()
    return nc


_CACHE = {}


F32R_ATTN = "bf16"
F32R_MM = "bf16"


def _get_program(has_v_bias: bool, chain: int = 1):
    key = ("prog", has_v_bias, F32R_ATTN, F32R_MM, chain)
    if key not in _CACHE:
        _CACHE[key] = _build_program(has_v_bias, F32R_ATTN, F32R_MM, chain)
    return _CACHE[key]


def _make_in_maps(x, gn_w, gn_b, qkv_w, qkv_b, proj_w):
    x = np.ascontiguousarray(x, dtype=np.float32)
    in_maps = []
    for core in range(NCORES):
        b, p = core // 2, core % 2
        rows_q = slice(p * 128, (p + 1) * 128)
        rows_k = slice(256 + p * 128, 256 + (p + 1) * 128)
        rows_v = slice(512 + p * 128, 512 + (p + 1) * 128)
        m = {
            "xb": np.ascontiguousarray(x[b].reshape(C, N)),
            "wqkT": np.ascontiguousarray(
                np.concatenate([qkv_w[rows_q], qkv_w[rows_k]], axis=0).T.astype(np.float32)),
            "wvT": np.ascontiguousarray(qkv_w[rows_v].T.astype(np.float32)),
            "wpT": np.ascontiguousarray(proj_w[:, p * 128:(p + 1) * 128].T.astype(np.float32)),
            "gnw": np.ascontiguousarray(gn_w.astype(np.float32)),
            "gnb": np.ascontiguousarray(gn_b.astype(np.float32)),
            "qkb": np.ascontiguousarray(
                np.concatenate([qkv_b[rows_q], qkv_b[rows_k]]).astype(np.float32)),
        }
        if np.any(qkv_b[512:768]):
            m["vb"] = np.ascontiguousarray(qkv_b[rows_v].astype(np.float32))
        in_maps.append(m)
    return in_maps


def _get_executor(nc, donate=True):
    """Build (once) a cached jitted 8-core executor for the program.

    Mirrors concourse.bass2jax.run_bass_via_pjrt, but caches the jitted
    callable so repeat kernel() calls don't re-trace/re-compile the XLA
    wrapper.  Returns (fn, in_names, out_names) where fn takes a list of
    per-core input dicts and returns a list of per-core output dicts.
    """
    key = ("exec", id(nc), donate)
    if key in _CACHE:
        return _CACHE[key]
    import jax
    import concourse.mybir as _mybir
    from jax.experimental.shard_map import shard_map
    from jax.sharding import Mesh, PartitionSpec
    from concourse import bass2jax

    bass2jax.install_neuronx_cc_hook()
    partition_name = nc.partition_id_tensor.name if nc.partition_id_tensor else None
    in_names, out_names, out_avals, zero_outs = [], [], [], []
    for alloc in nc.m.functions[0].allocations:
        if not isinstance(alloc, _mybir.MemoryLocationSet):
            continue
        name = alloc.memorylocations[0].name
        if alloc.kind == "ExternalInput":
            if name != partition_name:
                in_names.append(name)
        elif alloc.kind == "ExternalOutput":
            shape = tuple(alloc.tensor_shape)
            dtype = _mybir.dt.np(alloc.dtype)
            out_names.append(name)
            out_avals.append(jax.core.ShapedArray(shape, dtype))
            zero_outs.append(np.zeros(shape, dtype))
    n_params = len(in_names)
    n_outs = len(out_avals)
    all_names = in_names + out_names + ([partition_name] if partition_name else [])

    def _body(*args):
        operands = list(args)
        if partition_name is not None:
            operands.append(bass2jax.partition_id_tensor())
        return tuple(bass2jax._bass_exec_p.bind(
            *operands,
            out_avals=tuple(out_avals),
            in_names=tuple(all_names),
            out_names=tuple(out_names),
            lowering_input_output_aliases=(),
            sim_require_finite=True,
            sim_require_nnan=True,
            nc=nc,
        ))

    devices = jax.devices()[:NCORES]
    mesh = Mesh(np.asarray(devices), ("core",))
    in_specs = (PartitionSpec("core"),) * (n_params + n_outs)
    out_specs = (PartitionSpec("core"),) * n_outs
    donate_idx = tuple(range(n_params, n_params + n_outs)) if donate else ()
    sharded = jax.jit(
        shard_map(_body, mesh=mesh, in_specs=in_specs, out_specs=out_specs,
                  check_rep=False),
        donate_argnums=donate_idx, keep_unused=True,
    )

    _CACHE[("sharded", id(nc))] = sharded
    _CACHE[("zeros", id(nc))] = [((NCORES * z.shape[0],) + z.shape[1:], z.dtype)
                                 for z in zero_outs]

    def fn(in_maps):
        concat_in = [
            np.concatenate([np.asarray(in_maps[c][nm]) for c in range(NCORES)], axis=0)
            for nm in in_names
        ]
        concat_zeros = [
            np.zeros((NCORES * z.shape[0], *z.shape[1:]), z.dtype) for z in zero_outs
        ]
        out_arrs = sharded(*concat_in, *concat_zeros)
        return [
            {nm: np.asarray(out_arrs[i]).reshape(NCORES, *out_avals[i].shape)[c]
             for i, nm in enumerate(out_names)}
            for c in range(NCORES)
        ]

    _CACHE[key] = (fn, in_names, out_names)
    return _CACHE[key]


def _prep(inputs):
    x = np.asarray(inputs["x"], dtype=np.float32)
    qkv_b = np.asarray(inputs["qkv_b"], dtype=np.float32)
    has_v_bias = bool(np.any(qkv_b[512:768]))
    nc = _get_program(has_v_bias)
    in_maps = _make_in_maps(
        x,
        np.asarray(inputs["gn_w"], dtype=np.float32),
        np.asarray(inputs["gn_b"], dtype=np.float32),
        np.asarray(inputs["qkv_w"], dtype=np.float32),
        qkv_b,
        np.asarray(inputs["proj_w"], dtype=np.float32),
    )
    return nc, in_maps, x


def run(inputs, trace=False):
    """Run the sharded kernel.  Returns (output, per-core results list)."""
    nc, in_maps, x = _prep(inputs)
    fn, _, _ = _get_executor(nc)
    results = fn(in_maps)
    proj_b = np.asarray(inputs["proj_b"], dtype=np.float32)
    parts = [results[c]["yp"] for c in range(NCORES)]
    y = np.stack([parts[2 * b] + parts[2 * b + 1] for b in range(B)])  # [B, C, N]
    y = y + proj_b[None, :, None]
    out = np.asarray(inputs["x"], dtype=np.float32) + y.reshape(B, C, 64, 64)
    return out.astype(np.float32), results


def _bench_callable(inputs, chain):
    """Device-resident, no-donate timing callable for one chain variant."""
    import jax
    qkv_b = np.asarray(inputs["qkv_b"], dtype=np.float32)
    has_v_bias = bool(np.any(qkv_b[512:768]))
    _, in_maps, _ = _prep(inputs)
    nc = _get_program(has_v_bias, chain=chain)
    key = ("exec", id(nc), False)
    fresh = key not in _CACHE
    _get_executor(nc, donate=False)
    fn, in_names, out_names = _CACHE[key]
    # reach into the jitted callable: rebuild device-resident args
    import concourse.mybir as _mybir
    sharded = _CACHE[("sharded", id(nc))]
    concat_in = _CACHE.get(("dev_in", id(nc)))
    if concat_in is None:
        arrs = []
        for nm in in_names:
            arrs.append(np.concatenate(
                [np.asarray(in_maps[c][nm]) for c in range(NCORES)], axis=0))
        zshapes = _CACHE[("zeros", id(nc))]
        for z in zshapes:
            arrs.append(np.zeros(z[0], z[1]))
        concat_in = [jax.device_put(a) for a in arrs]
        _CACHE[("dev_in", id(nc))] = concat_in

    def call():
        out = sharded(*concat_in)
        jax.block_until_ready(out)

    return call


def bench(inputs, n=25, reps=10):
    """Marginal per-execution device time via an n-times-chained NEFF."""
    import time as _time
    f1 = _bench_callable(inputs, 1)
    fn = _bench_callable(inputs, n)

    def timed(f):
        f()
        best = float("inf")
        for _ in range(reps):
            t0 = _time.perf_counter()
            f()
            best = min(best, _time.perf_counter() - t0)
        return best

    t1, tn = timed(f1), timed(fn)
    return (tn - t1) / (n - 1), t1, tn


def kernel(**inputs) -> np.ndarray:
    out, _ = run(inputs, trace=False)
    return out

